# Initial kernel scaffold
#
"""GCN (2-layer GraphConv + ReLU + log_softmax) on 8 Trainium2 NeuronCores.

Strategy (graph/data parallel, per sharding hint):
  - Nodes are padded to 50176 = 8 * 49 * 128 and sharded contiguously:
    core c owns nodes [c*6272, (c+1)*6272).
  - Edges are routed to the core owning their *destination* node, sorted by
    destination tile (128 nodes), split into A/B groups by source index
    (src < 32768 vs >= 32768, so gather indices fit int16), and padded to
    128-edge chunks per (tile, group).
  - Layer 1 per core: dma_gather x[src] rows (256B each) from a replicated
    x table in HBM -> per-chunk one-hot selector (DVE is_equal against an
    iota row) -> PE matmul segment-sum accumulating aggT[64,128] in PSUM
    per destination tile -> projections with host-transposed weights ->
    ReLU -> hT[128,128].
  - The layer-2 "rel" projection is applied *before* the second gather
    (segment_sum is linear): hp = h @ W2_rel.T is computed per tile,
    written to a DRAM bounce buffer, and AllGathered across the 8 cores
    (1.6MB/rank). Layer-2 root term hroot = h @ W2_root.T + b2 stays
    resident in SBUF.
  - Layer 2 per core: dma_gather hp[src] rows from the AllGathered table,
    same selector/matmul segment-sum into agg2[128,40], add hroot,
    log_softmax along classes, DMA out.
"""

import math

import numpy as np

# ---------------------------------------------------------------- problem cfg

P = 128


class Cfg:
    def __init__(self, n, e, in_ch, hid, out_ch, n_cores, tiles_per_core, split,
                 seg_tiles=7):
        self.N = n
        self.E = e
        self.IN = in_ch
        self.HID = hid
        self.OUT = out_ch
        self.NCORES = n_cores
        self.TPC = tiles_per_core
        self.NPC = tiles_per_core * P
        self.NPAD = self.NPC * n_cores
        self.SPLIT = split
        self.SEG = seg_tiles
        assert self.NPAD >= self.N
        assert self.SPLIT % P == 0
        assert self.IN * 4 % 256 == 0  # dma_gather elem constraint


REAL_CFG = Cfg(n=50000, e=800000, in_ch=64, hid=128, out_ch=40, n_cores=8,
               tiles_per_core=49, split=32768, seg_tiles=7)


# ---------------------------------------------------------------- host preproc

def preprocess(x, edge_index, cfg):
    """Build per-core gather-index / selector tensors and the chunk structure.

    Returns (kA, kB, per_core) where kA/kB are per-tile chunk counts (uniform
    across cores; padded to the max) and per_core is a list of dicts of numpy
    arrays for each core's in_map.
    """
    N, E = cfg.N, cfg.E
    src = np.asarray(edge_index[0], dtype=np.int64)
    dst = np.asarray(edge_index[1], dtype=np.int64)

    gtile = dst // P                      # global dst tile in [0, NCORES*TPC)
    grp = (src >= cfg.SPLIT).astype(np.int64)
    key = gtile * 2 + grp
    order = np.argsort(key, kind="stable")
    src_s = src[order]
    key_s = key[order]

    nkeys = cfg.NCORES * cfg.TPC * 2
    bounds = np.searchsorted(key_s, np.arange(nkeys + 1))
    counts = np.diff(bounds).reshape(cfg.NCORES, cfg.TPC, 2)

    kA = np.maximum(np.ceil(counts[:, :, 0] / P).max(axis=0), 0).astype(int)
    kB = np.maximum(np.ceil(counts[:, :, 1] / P).max(axis=0), 0).astype(int)
    # every tile gets at least one chunk so the PSUM accumulate chain exists
    kA = np.maximum(kA, 1)

    dst_rel_s = (dst[order] % P).astype(np.float32)

    def build_group(c, g, kX):
        """Concatenate this core's per-tile edge lists for group g, padding
        each tile to kX[t]*128 edges with (idx=0, dst_rel=-1)."""
        idx_parts = []
        rel_parts = []
        for t in range(cfg.TPC):
            kk = key_base = (c * cfg.TPC + t) * 2 + g
            lo, hi = bounds[key_base], bounds[key_base + 1]
            n_real = hi - lo
            n_slots = kX[t] * P
            assert n_real <= n_slots
            idx = np.zeros(n_slots, dtype=np.int16)
            rel = np.full(n_slots, -1.0, dtype=np.float32)
            if n_real:
                s = src_s[lo:hi]
                idx[:n_real] = (s - (cfg.SPLIT if g else 0)).astype(np.int16)
                rel[:n_real] = dst_rel_s[lo:hi]
            idx_parts.append(idx)
            rel_parts.append(rel)
        return np.concatenate(idx_parts), np.concatenate(rel_parts)

    def idx_layout(flat16):
        # dma_gather index layout: [128, n/16]; idx i at (i%16, i//16),
        # replicated across the 8 Q7 cores (partitions 16k+r == r).
        cols = flat16.reshape(-1, 16).T          # [16, cols]
        return np.ascontiguousarray(np.tile(cols, (8, 1)))  # [128, cols]

    def rel_layout(flat):
        return np.ascontiguousarray(flat.reshape(-1, P).T)  # [128, n_chunks]

    x_pad = np.zeros((cfg.NPAD, cfg.IN), dtype=np.float32)
    x_pad[:N] = np.asarray(x, dtype=np.float32)

    per_core = []
    for c in range(cfg.NCORES):
        idxA, relA = build_group(c, 0, kA)
        idxB, relB = build_group(c, 1, kB)
        xT_own = np.ascontiguousarray(
            x_pad[c * cfg.NPC:(c + 1) * cfg.NPC].T)   # [IN, NPC]
        per_core.append(dict(
            x_tab=x_pad,
            xT_own=xT_own,
            idxA=idx_layout(idxA),
            idxB=idx_layout(idxB),
            drA=rel_layout(relA),
            drB=rel_layout(relB),
        ))
    return list(kA), list(kB), per_core


def make_weight_inputs(W1_rel, b1, W1_root, W2_rel, b2, W2_root, cfg):
    f = np.float32
    return dict(
        w1relT=np.ascontiguousarray(np.asarray(W1_rel, f).T),    # [IN, HID]
        w1rootT=np.ascontiguousarray(np.asarray(W1_root, f).T),  # [IN, HID]
        b1=np.asarray(b1, f).reshape(cfg.HID, 1).copy(),
        w2relT=np.ascontiguousarray(np.asarray(W2_rel, f).T),    # [HID, OUT]
        w2rootT=np.ascontiguousarray(np.asarray(W2_root, f).T),  # [HID, OUT]
        b2rep=np.tile(np.asarray(b2, f).reshape(1, cfg.OUT), (P, 1)).copy(),
        iota=np.tile(np.arange(P, dtype=f).reshape(1, P), (P, 1)).copy(),
    )


# ---------------------------------------------------------------- bass build

def build(cfg, kA, kB, stage="full", reps=8):
    import concourse.bacc as bacc
    import concourse.tile as tile
    from concourse import mybir

    f32 = mybir.dt.float32
    i16 = mybir.dt.int16
    Alu = mybir.AluOpType
    Act = mybir.ActivationFunctionType

    IN, HID, OUT, TPC = cfg.IN, cfg.HID, cfg.OUT, cfg.TPC
    NAc, NBc = sum(kA), sum(kB)
    offA = np.concatenate([[0], np.cumsum(kA)]).astype(int)
    offB = np.concatenate([[0], np.cumsum(kB)]).astype(int)
    n_tab_B = cfg.NPAD - cfg.SPLIT

    nc = bacc.Bacc("TRN2", target_bir_lowering=False, debug=False,
                   num_devices=cfg.NCORES)

    x_tab = nc.dram_tensor("x_tab", [cfg.NPAD, IN], f32, kind="ExternalInput")
    xT_own_d = nc.dram_tensor("xT_own", [IN, cfg.NPC], f32, kind="ExternalInput")
    idxA_d = nc.dram_tensor("idxA", [P, NAc * 8], i16, kind="ExternalInput")
    idxB_d = nc.dram_tensor("idxB", [P, NBc * 8], i16, kind="ExternalInput")
    drA_d = nc.dram_tensor("drA", [P, NAc], f32, kind="ExternalInput")
    drB_d = nc.dram_tensor("drB", [P, NBc], f32, kind="ExternalInput")
    w1relT_d = nc.dram_tensor("w1relT", [IN, HID], f32, kind="ExternalInput")
    w1rootT_d = nc.dram_tensor("w1rootT", [IN, HID], f32, kind="ExternalInput")
    b1_d = nc.dram_tensor("b1", [HID, 1], f32, kind="ExternalInput")
    w2relT_d = nc.dram_tensor("w2relT", [HID, OUT], f32, kind="ExternalInput")
    w2rootT_d = nc.dram_tensor("w2rootT", [HID, OUT], f32, kind="ExternalInput")
    b2rep_d = nc.dram_tensor("b2rep", [P, OUT], f32, kind="ExternalInput")
    iota_d = nc.dram_tensor("iota", [P, P], f32, kind="ExternalInput")
    out_d = nc.dram_tensor("out", [cfg.NPC, OUT], f32, kind="ExternalOutput")

    segs = [(s, min(s + cfg.SEG, TPC)) for s in range(0, TPC, cfg.SEG)]

    with tile.TileContext(nc) as tc:
        with (
            tc.tile_pool(name="const", bufs=1) as cp,
            tc.tile_pool(name="dram", bufs=1, space="DRAM") as dp,
        ):
            # ---- resident constants
            iota_s = cp.tile([P, P], f32)
            nc.sync.dma_start(iota_s[:], iota_d[:])
            w1relT_s = cp.tile([IN, HID], f32)
            nc.sync.dma_start(w1relT_s[:], w1relT_d[:])
            w1rootT_s = cp.tile([IN, HID], f32)
            nc.sync.dma_start(w1rootT_s[:], w1rootT_d[:])
            b1_s = cp.tile([HID, 1], f32)
            nc.sync.dma_start(b1_s[:], b1_d[:])
            w2relT_s = cp.tile([HID, OUT], f32)
            nc.sync.dma_start(w2relT_s[:], w2relT_d[:])
            w2rootT_s = cp.tile([HID, OUT], f32)
            nc.sync.dma_start(w2rootT_s[:], w2rootT_d[:])
            b2_s = cp.tile([P, OUT], f32)
            nc.sync.dma_start(b2_s[:], b2rep_d[:])
            xT_own_s = cp.tile([IN, cfg.NPC], f32)
            nc.sync.dma_start(xT_own_s[:], xT_own_d[:])
            idxA_s = cp.tile([P, NAc * 8], i16)
            nc.sync.dma_start(idxA_s[:], idxA_d[:])
            idxB_s = cp.tile([P, NBc * 8], i16)
            nc.sync.dma_start(idxB_s[:], idxB_d[:])
            drA_s = cp.tile([P, NAc], f32)
            nc.sync.dma_start(drA_s[:], drA_d[:])
            drB_s = cp.tile([P, NBc], f32)
            nc.sync.dma_start(drB_s[:], drB_d[:])
            hroots = cp.tile([P, TPC * OUT], f32)

            hp_local = dp.tile([cfg.NPC, IN], f32)
            hp_full = dp.tile([cfg.NPAD, IN], f32)

            maxA = max(offA[t1] - offA[t0] for t0, t1 in segs)
            maxB = max(offB[t1] - offB[t0] for t0, t1 in segs)

            def layer(phase, gtabA, gtabB, consume_tile):
                """One gather+segment-sum pass over all tiles."""
                with (
                    tc.tile_pool(name=f"G{phase}", bufs=2) as gp,
                    tc.tile_pool(name=f"S{phase}", bufs=4) as sp,
                    tc.tile_pool(name=f"agg{phase}", bufs=2, space="PSUM") as ap,
                ):
                    for t0, t1 in segs:
                        a0, a1 = offA[t0], offA[t1]
                        b0, b1_ = offB[t0], offB[t1]
                        nA, nB = a1 - a0, b1_ - b0
                        # dma_gather fails above 1024 indices per call
                        # (HW ring limit) -- split into <=8-chunk pieces.
                        GMAX = 8
                        GA = gp.tile([P, maxA, IN], f32, tag="GA")
                        for c0 in range(0, nA, GMAX):
                            c1 = min(c0 + GMAX, nA)
                            nc.gpsimd.dma_gather(
                                GA[:, c0:c1, :], gtabA,
                                idxA_s[:, (a0 + c0) * 8:(a0 + c1) * 8],
                                (c1 - c0) * P, (c1 - c0) * P, IN)
                        GB = gp.tile([P, maxB, IN], f32, tag="GB")
                        for c0 in range(0, nB, GMAX):
                            c1 = min(c0 + GMAX, nB)
                            nc.gpsimd.dma_gather(
                                GB[:, c0:c1, :], gtabB,
                                idxB_s[:, (b0 + c0) * 8:(b0 + c1) * 8],
                                (c1 - c0) * P, (c1 - c0) * P, IN)
                        kmaxA = max(kA)
                        kmaxB = max(kB) if max(kB) else 1
                        for t in range(t0, t1):
                            nch = kA[t] + kB[t]
                            if phase == 1:
                                acc = ap.tile([IN, P], f32, tag="acc")
                            else:
                                acc = ap.tile([P, OUT], f32, tag="acc")
                            ci = 0
                            for g, G, off, soff, dr, kmax in (
                                (0, GA, offA[t] - a0, offA[t], drA_s, kmaxA),
                                (1, GB, offB[t] - b0, offB[t], drB_s, kmaxB),
                            ):
                                kk = kA[t] if g == 0 else kB[t]
                                if kk == 0:
                                    continue
                                S = sp.tile([P, kmax, P], f32, tag=f"S{g}")
                                nc.vector.tensor_tensor(
                                    out=S[:, :kk, :],
                                    in0=iota_s[:].unsqueeze(1)
                                        .to_broadcast([P, kk, P]),
                                    in1=dr[:, soff:soff + kk].unsqueeze(2)
                                        .to_broadcast([P, kk, P]),
                                    op=Alu.is_equal)
                                for j in range(kk):
                                    if phase == 1:
                                        # aggT[f, d] += sum_e G[e,f] S[e,d]
                                        nc.tensor.matmul(
                                            acc[:], lhsT=G[:, off + j, :],
                                            rhs=S[:, j, :],
                                            start=(ci == 0),
                                            stop=(ci == nch - 1))
                                    else:
                                        # agg2[d, o] += sum_e S[e,d] G[e,:OUT]
                                        nc.tensor.matmul(
                                            acc[:], lhsT=S[:, j, :],
                                            rhs=G[:, off + j, :OUT],
                                            start=(ci == 0),
                                            stop=(ci == nch - 1))
                                    ci += 1
                            consume_tile(t, acc)

            # ---------------- phase 1
            sb1_cm = tc.tile_pool(name="sb1", bufs=3)
            hps_cm = tc.tile_pool(name="hps", bufs=2, space="PSUM")
            sb1 = sb1_cm.__enter__()
            hps = hps_cm.__enter__() if stage != "agg" else None

            def consume1(t, acc):
                aggsb = sb1.tile([IN, P], f32, tag="aggsb")
                nc.vector.tensor_copy(out=aggsb[:], in_=acc[:])
                if stage == "agg":
                    nc.sync.dma_start(
                        out=out_d[t * P:t * P + IN, :OUT],
                        in_=aggsb[:, :OUT])
                    return
                hT_ps = hps.tile([HID, P], f32, tag="hT")
                nc.tensor.matmul(hT_ps[:], lhsT=w1relT_s[:], rhs=aggsb[:],
                                 start=True, stop=False)
                nc.tensor.matmul(hT_ps[:], lhsT=w1rootT_s[:],
                                 rhs=xT_own_s[:, t * P:(t + 1) * P],
                                 start=False, stop=True)
                hT_sb = sb1.tile([HID, P], f32, tag="hTsb")
                nc.scalar.activation(hT_sb[:], hT_ps[:], Act.Relu,
                                     bias=b1_s[:, 0:1])
                if stage == "proj":
                    nc.sync.dma_start(
                        out=out_d[t * P:(t + 1) * P, :OUT],
                        in_=hT_sb[:, :OUT])
                    return
                hh_ps = hps.tile([P, 2 * OUT], f32, tag="hh")
                nc.tensor.matmul(hh_ps[:, :OUT], lhsT=hT_sb[:],
                                 rhs=w2relT_s[:], start=True, stop=True)
                nc.tensor.matmul(hh_ps[:, OUT:], lhsT=hT_sb[:],
                                 rhs=w2rootT_s[:], start=True, stop=True)
                hp_sb = sb1.tile([P, OUT], f32, tag="hpsb")
                nc.scalar.activation(hp_sb[:], hh_ps[:, :OUT], Act.Copy)
                nc.vector.tensor_tensor(
                    out=hroots[:, t * OUT:(t + 1) * OUT],
                    in0=hh_ps[:, OUT:], in1=b2_s[:], op=Alu.add)
                nc.sync.dma_start(
                    out=hp_local[t * P:(t + 1) * P, :OUT],
                    in_=hp_sb[:])

            layer(1, x_tab[:cfg.SPLIT, :], x_tab[cfg.SPLIT:, :], consume1)
            if hps is not None and stage != "time":
                hps_cm.__exit__(None, None, None)

            if stage == "p1":
                for t in range(TPC):
                    r = sb1.tile([P, OUT], f32, tag="r1")
                    nc.vector.tensor_copy(
                        out=r[:], in_=hroots[:, t * OUT:(t + 1) * OUT])
                    nc.sync.dma_start(out=out_d[t * P:(t + 1) * P, :],
                                      in_=r[:])
                sb1_cm.__exit__(None, None, None)
            if stage in ("coll", "full", "time"):
                # ---------------- allgather hp
                nc.gpsimd.collective_compute(
                    "AllGather", mybir.AluOpType.bypass,
                    replica_groups=[list(range(cfg.NCORES))],
                    ins=[hp_local[:cfg.NPC, :]],
                    outs=[hp_full[:cfg.NPAD, :]],
                )

            if stage == "coll":
                for t in range(TPC):
                    r = sb1.tile([P, OUT], f32, tag="r1")
                    nc.sync.dma_start(
                        out=r[:], in_=hp_full[t * P:(t + 1) * P, :OUT])
                    nc.sync.dma_start(out=out_d[t * P:(t + 1) * P, :],
                                      in_=r[:])
                sb1_cm.__exit__(None, None, None)

            # ---------------- phase 2
            sb2_cm = tc.tile_pool(name="sb2", bufs=3)
            sb2 = sb2_cm.__enter__()

            def consume2(t, acc):
                o1 = sb2.tile([P, OUT], f32, tag="o1")
                nc.vector.tensor_tensor(
                    out=o1[:], in0=acc[:],
                    in1=hroots[:, t * OUT:(t + 1) * OUT], op=Alu.add)
                mx = sb2.tile([P, 1], f32, tag="mx")
                nc.vector.reduce_max(out=mx[:], in_=o1[:],
                                     axis=mybir.AxisListType.X)
                nmx = sb2.tile([P, 1], f32, tag="nmx")
                nc.vector.tensor_scalar(nmx[:], mx[:], -1.0, None, Alu.mult)
                esc = sb2.tile([P, OUT], f32, tag="esc")
                ssum = sb2.tile([P, 1], f32, tag="ssum")
                nc.scalar.activation(esc[:], o1[:], Act.Exp,
                                     bias=nmx[:, 0:1], accum_out=ssum[:])
                lse = sb2.tile([P, 1], f32, tag="lse")
                nc.scalar.activation(lse[:], ssum[:], Act.Ln)
                shift = sb2.tile([P, 1], f32, tag="shift")
                nc.vector.tensor_tensor(out=shift[:], in0=mx[:], in1=lse[:],
                                        op=Alu.add)
                res = sb2.tile([P, OUT], f32, tag="res")
                nc.vector.tensor_tensor(
                    out=res[:], in0=o1[:],
                    in1=shift[:, 0:1].to_broadcast([P, OUT]),
                    op=Alu.subtract)
                nc.sync.dma_start(out=out_d[t * P:(t + 1) * P, :], in_=res[:])

            if stage == "full":
                layer(2, hp_full[:cfg.SPLIT, :], hp_full[cfg.SPLIT:, :],
                      consume2)
            elif stage == "time":
                layer(2, hp_full[:cfg.SPLIT, :], hp_full[cfg.SPLIT:, :],
                      consume2)
                # repeat both phases (minus the collective) to measure pure
                # kernel time as the wall-clock delta per extra rep
                with tc.For_i(0, reps, 1):
                    layer(1, x_tab[:cfg.SPLIT, :], x_tab[cfg.SPLIT:, :],
                          consume1)
                    layer(2, hp_full[:cfg.SPLIT, :], hp_full[cfg.SPLIT:, :],
                          consume2)
            sb2_cm.__exit__(None, None, None)
            if stage == "time":
                hps_cm.__exit__(None, None, None)
            if stage in ("full", "time", "agg", "proj"):
                sb1_cm.__exit__(None, None, None)

    nc.compile()
    return nc


# ---------------------------------------------------------------- runner

_CACHE = {}


def _get_program(cfg, kA, kB):
    key = (cfg.N, cfg.E, cfg.NCORES, cfg.TPC, tuple(kA), tuple(kB))
    if key not in _CACHE:
        _CACHE[key] = build(cfg, kA, kB)
    return _CACHE[key]


def run_gcn(inputs, cfg, trace=False):
    from concourse import bass_utils

    kA, kB, per_core = preprocess(inputs["x"], inputs["edge_index"], cfg)
    wts = make_weight_inputs(inputs["W1_rel"], inputs["b1"], inputs["W1_root"],
                             inputs["W2_rel"], inputs["b2"], inputs["W2_root"],
                             cfg)
    nc = _get_program(cfg, kA, kB)
    in_maps = []
    for c in range(cfg.NCORES):
        m = dict(per_core[c])
        m.update(wts)
        # rename to dram tensor names
        in_maps.append({
            "x_tab": m["x_tab"], "xT_own": m["xT_own"],
            "idxA": m["idxA"], "idxB": m["idxB"],
            "drA": m["drA"], "drB": m["drB"],
            "w1relT": m["w1relT"], "w1rootT": m["w1rootT"], "b1": m["b1"],
            "w2relT": m["w2relT"], "w2rootT": m["w2rootT"],
            "b2rep": m["b2rep"], "iota": m["iota"],
        })
    res = bass_utils.run_bass_kernel_spmd(
        nc, in_maps, core_ids=list(range(cfg.NCORES)), trace=trace)
    outs = [res.results[c]["out"] for c in range(cfg.NCORES)]
    full = np.concatenate(outs, axis=0)[:cfg.N]
    return full, res


def kernel(**inputs):
    out, _ = run_gcn(inputs, REAL_CFG)
    return out



# revision 15
# speedup vs baseline: 2.0904x; 2.0904x over previous
"""GCN (2-layer GraphConv + ReLU + log_softmax) on 8 Trainium2 NeuronCores.

Strategy (graph/data parallel, per sharding hint):
  - Nodes are padded to 50176 = 8 * 49 * 128 and sharded contiguously:
    core c owns nodes [c*6272, (c+1)*6272).
  - Edges are routed to the core owning their *destination* node, sorted by
    destination tile (128 nodes), split into A/B groups by source index
    (src < 32768 vs >= 32768, so gather indices fit int16), and padded to
    128-edge chunks per (tile, group).
  - Layer 1 per core: dma_gather x[src] rows (256B each) from a replicated
    x table in HBM -> per-chunk one-hot selector (DVE is_equal against an
    iota row) -> PE matmul segment-sum accumulating aggT[64,128] in PSUM
    per destination tile -> projections with host-transposed weights ->
    ReLU -> hT[128,128].
  - The layer-2 "rel" projection is applied *before* the second gather
    (segment_sum is linear): hp = h @ W2_rel.T is computed per tile,
    written to a DRAM bounce buffer, and AllGathered across the 8 cores
    (1.6MB/rank). Layer-2 root term hroot = h @ W2_root.T + b2 stays
    resident in SBUF.
  - Layer 2 per core: dma_gather hp[src] rows from the AllGathered table,
    same selector/matmul segment-sum into agg2[128,40], add hroot,
    log_softmax along classes, DMA out.
"""

import math

import numpy as np

# ---------------------------------------------------------------- problem cfg

P = 128


class Cfg:
    def __init__(self, n, e, in_ch, hid, out_ch, n_cores, tiles_per_core, split,
                 seg_tiles=7):
        self.N = n
        self.E = e
        self.IN = in_ch
        self.HID = hid
        self.OUT = out_ch
        self.NCORES = n_cores
        self.TPC = tiles_per_core
        self.NPC = tiles_per_core * P
        self.NPAD = self.NPC * n_cores
        self.SPLIT = split
        self.SEG = seg_tiles
        assert self.NPAD >= self.N
        assert self.SPLIT % P == 0
        assert self.IN * 4 % 256 == 0  # dma_gather elem constraint


REAL_CFG = Cfg(n=50000, e=800000, in_ch=64, hid=128, out_ch=40, n_cores=8,
               tiles_per_core=49, split=32768, seg_tiles=7)


# ---------------------------------------------------------------- host preproc

def preprocess(x, edge_index, cfg):
    """Build per-core gather-index / selector tensors and the chunk structure.

    Returns (kA, kB, per_core) where kA/kB are per-tile chunk counts (uniform
    across cores; padded to the max) and per_core is a list of dicts of numpy
    arrays for each core's in_map.
    """
    N, E = cfg.N, cfg.E
    src = np.asarray(edge_index[0], dtype=np.int64)
    dst = np.asarray(edge_index[1], dtype=np.int64)

    gtile = dst // P                      # global dst tile in [0, NCORES*TPC)
    grp = (src >= cfg.SPLIT).astype(np.int64)
    key = gtile * 2 + grp
    order = np.argsort(key, kind="stable")
    src_s = src[order]
    key_s = key[order]

    nkeys = cfg.NCORES * cfg.TPC * 2
    bounds = np.searchsorted(key_s, np.arange(nkeys + 1))
    counts = np.diff(bounds).reshape(cfg.NCORES, cfg.TPC, 2)

    kA = np.maximum(np.ceil(counts[:, :, 0] / P).max(axis=0), 0).astype(int)
    kB = np.maximum(np.ceil(counts[:, :, 1] / P).max(axis=0), 0).astype(int)
    # every tile gets at least one chunk so the PSUM accumulate chain exists
    kA = np.maximum(kA, 1)

    dst_rel_s = (dst[order] % P).astype(np.float32)

    def build_group(c, g, kX):
        """Concatenate this core's per-tile edge lists for group g, padding
        each tile to kX[t]*128 edges with (idx=0, dst_rel=-1)."""
        idx_parts = []
        rel_parts = []
        for t in range(cfg.TPC):
            kk = key_base = (c * cfg.TPC + t) * 2 + g
            lo, hi = bounds[key_base], bounds[key_base + 1]
            n_real = hi - lo
            n_slots = kX[t] * P
            assert n_real <= n_slots
            idx = np.zeros(n_slots, dtype=np.int16)
            rel = np.full(n_slots, -1.0, dtype=np.float32)
            if n_real:
                s = src_s[lo:hi]
                idx[:n_real] = (s - (cfg.SPLIT if g else 0)).astype(np.int16)
                rel[:n_real] = dst_rel_s[lo:hi]
            idx_parts.append(idx)
            rel_parts.append(rel)
        return np.concatenate(idx_parts), np.concatenate(rel_parts)

    def idx_layout(flat16):
        # dma_gather index layout: [128, n/16]; idx i at (i%16, i//16),
        # replicated across the 8 Q7 cores (partitions 16k+r == r).
        cols = flat16.reshape(-1, 16).T          # [16, cols]
        return np.ascontiguousarray(np.tile(cols, (8, 1)))  # [128, cols]

    def rel_layout(flat):
        return np.ascontiguousarray(flat.reshape(-1, P).T)  # [128, n_chunks]

    x_pad = np.zeros((cfg.NPAD, cfg.IN), dtype=np.float32)
    x_pad[:N] = np.asarray(x, dtype=np.float32)

    per_core = []
    for c in range(cfg.NCORES):
        idxA, relA = build_group(c, 0, kA)
        idxB, relB = build_group(c, 1, kB)
        xT_own = np.ascontiguousarray(
            x_pad[c * cfg.NPC:(c + 1) * cfg.NPC].T)   # [IN, NPC]
        per_core.append(dict(
            x_tab=x_pad,
            xT_own=xT_own,
            idxA=idx_layout(idxA),
            idxB=idx_layout(idxB),
            drA=rel_layout(relA),
            drB=rel_layout(relB),
        ))
    return list(kA), list(kB), per_core


def make_weight_inputs(W1_rel, b1, W1_root, W2_rel, b2, W2_root, cfg):
    f = np.float32
    return dict(
        w1relT=np.ascontiguousarray(np.asarray(W1_rel, f).T),    # [IN, HID]
        w1rootT=np.ascontiguousarray(np.asarray(W1_root, f).T),  # [IN, HID]
        b1=np.asarray(b1, f).reshape(cfg.HID, 1).copy(),
        w2relT=np.ascontiguousarray(np.asarray(W2_rel, f).T),    # [HID, OUT]
        w2rootT=np.ascontiguousarray(np.asarray(W2_root, f).T),  # [HID, OUT]
        b2rep=np.tile(np.asarray(b2, f).reshape(1, cfg.OUT), (P, 1)).copy(),
        iota=np.tile(np.arange(P, dtype=f).reshape(1, P), (P, 1)).copy(),
    )


# ---------------------------------------------------------------- bass build

def build(cfg, kA, kB, stage="full", reps=8):
    import concourse.bacc as bacc
    import concourse.tile as tile
    from concourse import mybir

    f32 = mybir.dt.float32
    f32r = mybir.dt.float32r
    i16 = mybir.dt.int16
    Alu = mybir.AluOpType
    Act = mybir.ActivationFunctionType

    IN, HID, OUT, TPC = cfg.IN, cfg.HID, cfg.OUT, cfg.TPC
    NAc, NBc = sum(kA), sum(kB)
    offA = np.concatenate([[0], np.cumsum(kA)]).astype(int)
    offB = np.concatenate([[0], np.cumsum(kB)]).astype(int)
    n_tab_B = cfg.NPAD - cfg.SPLIT

    nc = bacc.Bacc("TRN2", target_bir_lowering=False, debug=False,
                   num_devices=cfg.NCORES, num_swdge_queues=4)

    # f32r: same bits as f32, but the PE runs single-pass (fp32 weights are
    # dual-passed hi/lo) -- 2.4x faster agg matmuls at ~1e-4 rel err.
    x_tab = nc.dram_tensor("x_tab", [cfg.NPAD, IN], f32r, kind="ExternalInput")
    xT_own_d = nc.dram_tensor("xT_own", [IN, cfg.NPC], f32, kind="ExternalInput")
    idxA_d = nc.dram_tensor("idxA", [P, NAc * 8], i16, kind="ExternalInput")
    idxB_d = nc.dram_tensor("idxB", [P, NBc * 8], i16, kind="ExternalInput")
    drA_d = nc.dram_tensor("drA", [P, NAc], f32, kind="ExternalInput")
    drB_d = nc.dram_tensor("drB", [P, NBc], f32, kind="ExternalInput")
    w1relT_d = nc.dram_tensor("w1relT", [IN, HID], f32, kind="ExternalInput")
    w1rootT_d = nc.dram_tensor("w1rootT", [IN, HID], f32, kind="ExternalInput")
    b1_d = nc.dram_tensor("b1", [HID, 1], f32, kind="ExternalInput")
    w2relT_d = nc.dram_tensor("w2relT", [HID, OUT], f32, kind="ExternalInput")
    w2rootT_d = nc.dram_tensor("w2rootT", [HID, OUT], f32, kind="ExternalInput")
    b2rep_d = nc.dram_tensor("b2rep", [P, OUT], f32, kind="ExternalInput")
    iota_d = nc.dram_tensor("iota", [P, P], f32, kind="ExternalInput")
    out_d = nc.dram_tensor("out", [cfg.NPC, OUT], f32, kind="ExternalOutput")

    segs = [(s, min(s + cfg.SEG, TPC)) for s in range(0, TPC, cfg.SEG)]

    with tile.TileContext(nc) as tc:
        with (
            tc.tile_pool(name="const", bufs=1) as cp,
            tc.tile_pool(name="dram", bufs=1, space="DRAM") as dp,
        ):
            # ---- resident constants
            iota_s = cp.tile([P, P], f32)
            nc.sync.dma_start(iota_s[:], iota_d[:])
            w1relT_s = cp.tile([IN, HID], f32)
            nc.sync.dma_start(w1relT_s[:], w1relT_d[:])
            w1rootT_s = cp.tile([IN, HID], f32)
            nc.sync.dma_start(w1rootT_s[:], w1rootT_d[:])
            b1_s = cp.tile([HID, 1], f32)
            nc.sync.dma_start(b1_s[:], b1_d[:])
            w2relT_s = cp.tile([HID, OUT], f32)
            nc.sync.dma_start(w2relT_s[:], w2relT_d[:])
            w2rootT_s = cp.tile([HID, OUT], f32)
            nc.sync.dma_start(w2rootT_s[:], w2rootT_d[:])
            b2_s = cp.tile([P, OUT], f32)
            nc.sync.dma_start(b2_s[:], b2rep_d[:])
            xT_own_s = cp.tile([IN, cfg.NPC], f32)
            nc.sync.dma_start(xT_own_s[:], xT_own_d[:])
            idxA_s = cp.tile([P, NAc * 8], i16)
            nc.sync.dma_start(idxA_s[:], idxA_d[:])
            idxB_s = cp.tile([P, NBc * 8], i16)
            nc.sync.dma_start(idxB_s[:], idxB_d[:])
            drA_s = cp.tile([P, NAc], f32)
            nc.sync.dma_start(drA_s[:], drA_d[:])
            drB_s = cp.tile([P, NBc], f32)
            nc.sync.dma_start(drB_s[:], drB_d[:])
            hroots = cp.tile([P, TPC * OUT], f32)

            hp_local = dp.tile([cfg.NPC, IN], f32)
            hp_full = dp.tile([cfg.NPAD, IN], f32)

            maxA = max(offA[t1] - offA[t0] for t0, t1 in segs)
            maxB = max(offB[t1] - offB[t0] for t0, t1 in segs)

            def layer(phase, gtabA, gtabB, consume_tile):
                """One gather+segment-sum pass over all tiles."""
                with (
                    tc.tile_pool(name=f"G{phase}", bufs=2) as gp,
                    tc.tile_pool(name=f"S{phase}", bufs=4) as sp,
                    tc.tile_pool(name=f"agg{phase}", bufs=2, space="PSUM") as ap,
                ):
                    gq = [0]

                    def gather_piece(G, gtab, idx_s, base, c0, c1):
                        # round-robin the 4 SWDGE queues: each queue's ring
                        # caps at 1024 descriptors, and with a single queue
                        # the gather op stalls on ring drain (8.3us/call);
                        # spread across queues the calls pipeline (2.9us).
                        nc.gpsimd.dma_gather(
                            G[:, c0:c1, :], gtab,
                            idx_s[:, (base + c0) * 8:(base + c1) * 8],
                            (c1 - c0) * P, (c1 - c0) * P, IN,
                            queue_num=gq[0] % 4)
                        gq[0] += 1

                    gdt = f32r if phase == 1 else f32
                    for t0, t1 in segs:
                        a0, a1 = offA[t0], offA[t1]
                        b0, b1_ = offB[t0], offB[t1]
                        nA, nB = a1 - a0, b1_ - b0
                        # dma_gather fails above 1024 indices per call
                        # (HW ring limit) -- split into <=8-chunk pieces.
                        GMAX = 8
                        GA = gp.tile([P, maxA, IN], gdt, tag="GA")
                        for c0 in range(0, nA, GMAX):
                            gather_piece(GA, gtabA, idxA_s, a0,
                                         c0, min(c0 + GMAX, nA))
                        GB = gp.tile([P, maxB, IN], gdt, tag="GB")
                        for c0 in range(0, nB, GMAX):
                            gather_piece(GB, gtabB, idxB_s, b0,
                                         c0, min(c0 + GMAX, nB))
                        kmaxA = max(kA)
                        kmaxB = max(kB) if max(kB) else 1
                        for t in range(t0, t1):
                            nch = kA[t] + kB[t]
                            if phase == 1:
                                acc = ap.tile([IN, P], f32, tag="acc")
                            else:
                                acc = ap.tile([P, OUT], f32, tag="acc")
                            ci = 0
                            for g, G, off, soff, dr, kmax in (
                                (0, GA, offA[t] - a0, offA[t], drA_s, kmaxA),
                                (1, GB, offB[t] - b0, offB[t], drB_s, kmaxB),
                            ):
                                kk = kA[t] if g == 0 else kB[t]
                                if kk == 0:
                                    continue
                                S = sp.tile([P, kmax, P], gdt, tag=f"S{g}")
                                nc.vector.tensor_tensor(
                                    out=S[:, :kk, :],
                                    in0=iota_s[:].unsqueeze(1)
                                        .to_broadcast([P, kk, P]),
                                    in1=dr[:, soff:soff + kk].unsqueeze(2)
                                        .to_broadcast([P, kk, P]),
                                    op=Alu.is_equal)
                                for j in range(kk):
                                    if phase == 1:
                                        # aggT[f, d] += sum_e G[e,f] S[e,d]
                                        nc.tensor.matmul(
                                            acc[:], lhsT=G[:, off + j, :],
                                            rhs=S[:, j, :],
                                            start=(ci == 0),
                                            stop=(ci == nch - 1))
                                    else:
                                        # agg2[d, o] += sum_e S[e,d] G[e,:OUT]
                                        nc.tensor.matmul(
                                            acc[:], lhsT=S[:, j, :],
                                            rhs=G[:, off + j, :OUT],
                                            start=(ci == 0),
                                            stop=(ci == nch - 1))
                                    ci += 1
                            consume_tile(t, acc)

            # ---------------- phase 1
            sb1_cm = tc.tile_pool(name="sb1", bufs=3)
            hps_cm = tc.tile_pool(name="hps", bufs=2, space="PSUM")
            sb1 = sb1_cm.__enter__()
            hps = hps_cm.__enter__() if stage != "agg" else None

            def consume1(t, acc):
                aggsb = sb1.tile([IN, P], f32, tag="aggsb")
                nc.vector.tensor_copy(out=aggsb[:], in_=acc[:])
                if stage == "agg":
                    nc.sync.dma_start(
                        out=out_d[t * P:t * P + IN, :OUT],
                        in_=aggsb[:, :OUT])
                    return
                hT_ps = hps.tile([HID, P], f32, tag="hT")
                nc.tensor.matmul(hT_ps[:], lhsT=w1relT_s[:], rhs=aggsb[:],
                                 start=True, stop=False)
                nc.tensor.matmul(hT_ps[:], lhsT=w1rootT_s[:],
                                 rhs=xT_own_s[:, t * P:(t + 1) * P],
                                 start=False, stop=True)
                hT_sb = sb1.tile([HID, P], f32, tag="hTsb")
                nc.scalar.activation(hT_sb[:], hT_ps[:], Act.Relu,
                                     bias=b1_s[:, 0:1])
                if stage == "proj":
                    nc.sync.dma_start(
                        out=out_d[t * P:(t + 1) * P, :OUT],
                        in_=hT_sb[:, :OUT])
                    return
                hh_ps = hps.tile([P, 2 * OUT], f32, tag="hh")
                nc.tensor.matmul(hh_ps[:, :OUT], lhsT=hT_sb[:],
                                 rhs=w2relT_s[:], start=True, stop=True)
                nc.tensor.matmul(hh_ps[:, OUT:], lhsT=hT_sb[:],
                                 rhs=w2rootT_s[:], start=True, stop=True)
                hp_sb = sb1.tile([P, OUT], f32, tag="hpsb")
                nc.scalar.activation(hp_sb[:], hh_ps[:, :OUT], Act.Copy)
                nc.vector.tensor_tensor(
                    out=hroots[:, t * OUT:(t + 1) * OUT],
                    in0=hh_ps[:, OUT:], in1=b2_s[:], op=Alu.add)
                nc.sync.dma_start(
                    out=hp_local[t * P:(t + 1) * P, :OUT],
                    in_=hp_sb[:])

            layer(1, x_tab[:cfg.SPLIT, :], x_tab[cfg.SPLIT:, :], consume1)
            if hps is not None and stage != "time":
                hps_cm.__exit__(None, None, None)

            if stage == "p1":
                for t in range(TPC):
                    r = sb1.tile([P, OUT], f32, tag="r1")
                    nc.vector.tensor_copy(
                        out=r[:], in_=hroots[:, t * OUT:(t + 1) * OUT])
                    nc.sync.dma_start(out=out_d[t * P:(t + 1) * P, :],
                                      in_=r[:])
                sb1_cm.__exit__(None, None, None)
            if stage in ("coll", "full", "time"):
                # ---------------- allgather hp
                nc.gpsimd.collective_compute(
                    "AllGather", mybir.AluOpType.bypass,
                    replica_groups=[list(range(cfg.NCORES))],
                    ins=[hp_local[:cfg.NPC, :]],
                    outs=[hp_full[:cfg.NPAD, :]],
                )

            if stage == "coll":
                for t in range(TPC):
                    r = sb1.tile([P, OUT], f32, tag="r1")
                    nc.sync.dma_start(
                        out=r[:], in_=hp_full[t * P:(t + 1) * P, :OUT])
                    nc.sync.dma_start(out=out_d[t * P:(t + 1) * P, :],
                                      in_=r[:])
                sb1_cm.__exit__(None, None, None)

            # ---------------- phase 2
            sb2_cm = tc.tile_pool(name="sb2", bufs=3)
            sb2 = sb2_cm.__enter__()

            def consume2(t, acc):
                o1 = sb2.tile([P, OUT], f32, tag="o1")
                nc.vector.tensor_tensor(
                    out=o1[:], in0=acc[:],
                    in1=hroots[:, t * OUT:(t + 1) * OUT], op=Alu.add)
                mx = sb2.tile([P, 1], f32, tag="mx")
                nc.vector.reduce_max(out=mx[:], in_=o1[:],
                                     axis=mybir.AxisListType.X)
                nmx = sb2.tile([P, 1], f32, tag="nmx")
                nc.vector.tensor_scalar(nmx[:], mx[:], -1.0, None, Alu.mult)
                esc = sb2.tile([P, OUT], f32, tag="esc")
                ssum = sb2.tile([P, 1], f32, tag="ssum")
                nc.scalar.activation(esc[:], o1[:], Act.Exp,
                                     bias=nmx[:, 0:1], accum_out=ssum[:])
                lse = sb2.tile([P, 1], f32, tag="lse")
                nc.scalar.activation(lse[:], ssum[:], Act.Ln)
                shift = sb2.tile([P, 1], f32, tag="shift")
                nc.vector.tensor_tensor(out=shift[:], in0=mx[:], in1=lse[:],
                                        op=Alu.add)
                res = sb2.tile([P, OUT], f32, tag="res")
                nc.vector.tensor_tensor(
                    out=res[:], in0=o1[:],
                    in1=shift[:, 0:1].to_broadcast([P, OUT]),
                    op=Alu.subtract)
                nc.sync.dma_start(out=out_d[t * P:(t + 1) * P, :], in_=res[:])

            if stage == "full":
                layer(2, hp_full[:cfg.SPLIT, :], hp_full[cfg.SPLIT:, :],
                      consume2)
            elif stage == "time":
                layer(2, hp_full[:cfg.SPLIT, :], hp_full[cfg.SPLIT:, :],
                      consume2)
                # repeat both phases (minus the collective) to measure pure
                # kernel time as the wall-clock delta per extra rep
                with tc.For_i(0, reps, 1):
                    layer(1, x_tab[:cfg.SPLIT, :], x_tab[cfg.SPLIT:, :],
                          consume1)
                    layer(2, hp_full[:cfg.SPLIT, :], hp_full[cfg.SPLIT:, :],
                          consume2)
            sb2_cm.__exit__(None, None, None)
            if stage == "time":
                hps_cm.__exit__(None, None, None)
            if stage in ("full", "time", "agg", "proj"):
                sb1_cm.__exit__(None, None, None)

    nc.compile()
    return nc


# ---------------------------------------------------------------- runner

_CACHE = {}


def _get_program(cfg, kA, kB):
    key = (cfg.N, cfg.E, cfg.NCORES, cfg.TPC, tuple(kA), tuple(kB))
    if key not in _CACHE:
        _CACHE[key] = build(cfg, kA, kB)
    return _CACHE[key]


def run_gcn(inputs, cfg, trace=False):
    from concourse import bass_utils

    kA, kB, per_core = preprocess(inputs["x"], inputs["edge_index"], cfg)
    wts = make_weight_inputs(inputs["W1_rel"], inputs["b1"], inputs["W1_root"],
                             inputs["W2_rel"], inputs["b2"], inputs["W2_root"],
                             cfg)
    nc = _get_program(cfg, kA, kB)
    in_maps = []
    for c in range(cfg.NCORES):
        m = dict(per_core[c])
        m.update(wts)
        # rename to dram tensor names
        in_maps.append({
            "x_tab": m["x_tab"], "xT_own": m["xT_own"],
            "idxA": m["idxA"], "idxB": m["idxB"],
            "drA": m["drA"], "drB": m["drB"],
            "w1relT": m["w1relT"], "w1rootT": m["w1rootT"], "b1": m["b1"],
            "w2relT": m["w2relT"], "w2rootT": m["w2rootT"],
            "b2rep": m["b2rep"], "iota": m["iota"],
        })
    res = bass_utils.run_bass_kernel_spmd(
        nc, in_maps, core_ids=list(range(cfg.NCORES)), trace=trace)
    outs = [res.results[c]["out"] for c in range(cfg.NCORES)]
    full = np.concatenate(outs, axis=0)[:cfg.N]
    return full, res


def kernel(**inputs):
    out, _ = run_gcn(inputs, REAL_CFG)
    return out



# revision 18
# speedup vs baseline: 2.3110x; 1.1055x over previous
"""GCN (2-layer GraphConv + ReLU + log_softmax) on 8 Trainium2 NeuronCores.

Strategy (graph/data parallel, per sharding hint):
  - Nodes are padded to 50176 = 8 * 49 * 128 and sharded contiguously:
    core c owns nodes [c*6272, (c+1)*6272).
  - Edges are routed to the core owning their *destination* node, sorted by
    destination tile (128 nodes), split into A/B groups by source index
    (src < 32768 vs >= 32768, so gather indices fit int16), and padded to
    128-edge chunks per (tile, group).
  - Gathered feature rows use a bf16 hi/lo split packed into one 256B row:
    row = [bf16(v) (64/40 cols) | bf16(v - hi) | pad]. The segment-sum
    matmul streams both halves through the PE in a single bf16 op
    (selector one-hot stationary, exact in bf16) and the two halves are
    re-added afterwards -- f32-quality precision (~2^-18 rel) at bf16
    matmul speed, with the same 256B/row gather wire cost as f32.
  - Gathers round-robin the 4 SWDGE queues (a single queue stalls the
    gather ucode on ring drain: 8.3us vs 2.9us per 1024-row call).
  - Layer 1 per core: dma_gather x[src] rows from the replicated packed x
    table in HBM -> per-chunk one-hot bf16 selector (DVE is_equal) ->
    one PE matmul per chunk accumulating aggT[128d, hi|lo] in PSUM ->
    hi+lo add -> PE transpose -> projections (f32) -> ReLU -> hT.
  - The layer-2 "rel" projection is applied *before* the second gather
    (segment_sum is linear): hp = h @ W2_rel.T per tile, split hi/lo bf16,
    packed to [NPC, 128] bf16, AllGathered (1.6MB/rank). Layer-2 root term
    hroot = h @ W2_root.T + b2 stays resident in SBUF.
  - Layer 2 per core: dma_gather hp rows, same selector matmul into
    acc2[128d, 40hi|40lo], hi+lo+hroot adds, log_softmax, DMA out.
"""

import numpy as np

# ---------------------------------------------------------------- problem cfg

P = 128


class Cfg:
    def __init__(self, n, e, in_ch, hid, out_ch, n_cores, tiles_per_core, split,
                 seg_tiles=7):
        self.N = n
        self.E = e
        self.IN = in_ch
        self.HID = hid
        self.OUT = out_ch
        self.NCORES = n_cores
        self.TPC = tiles_per_core
        self.NPC = tiles_per_core * P
        self.NPAD = self.NPC * n_cores
        self.SPLIT = split
        self.SEG = seg_tiles
        assert self.NPAD >= self.N
        assert self.SPLIT % P == 0


REAL_CFG = Cfg(n=50000, e=800000, in_ch=64, hid=128, out_ch=40, n_cores=8,
               tiles_per_core=49, split=32768, seg_tiles=7)


# ---------------------------------------------------------------- host preproc

def _bf16_round(x):
    """Round f32 -> bf16 (round-to-nearest-even), keep as float32 bits."""
    u = x.view(np.uint32)
    r = (u + 0x7FFF + ((u >> 16) & 1)) & 0xFFFF0000
    return r.view(np.float32)


def pack_hilo(v, width):
    """[R, C] f32 -> [R, width] bf16 rows [hi(C) | lo(C) | pad]."""
    import ml_dtypes
    hi = _bf16_round(np.ascontiguousarray(v))
    lo = _bf16_round(np.ascontiguousarray(v - hi))
    out = np.zeros((v.shape[0], width), dtype=ml_dtypes.bfloat16)
    out[:, :v.shape[1]] = hi.astype(ml_dtypes.bfloat16)
    out[:, v.shape[1]:2 * v.shape[1]] = lo.astype(ml_dtypes.bfloat16)
    return out


def preprocess(x, edge_index, cfg):
    """Build per-core gather-index / selector tensors and the chunk structure.

    Returns (kA, kB, per_core) where kA/kB are per-tile chunk counts (uniform
    across cores; padded to the max) and per_core is a list of dicts of numpy
    arrays for each core's in_map.
    """
    N, E = cfg.N, cfg.E
    src = np.asarray(edge_index[0], dtype=np.int64)
    dst = np.asarray(edge_index[1], dtype=np.int64)

    gtile = dst // P                      # global dst tile in [0, NCORES*TPC)
    grp = (src >= cfg.SPLIT).astype(np.int64)
    key = gtile * 2 + grp
    order = np.argsort(key, kind="stable")
    src_s = src[order]
    key_s = key[order]

    nkeys = cfg.NCORES * cfg.TPC * 2
    bounds = np.searchsorted(key_s, np.arange(nkeys + 1))
    counts = np.diff(bounds).reshape(cfg.NCORES, cfg.TPC, 2)

    kA = np.maximum(np.ceil(counts[:, :, 0] / P).max(axis=0), 0).astype(int)
    kB = np.maximum(np.ceil(counts[:, :, 1] / P).max(axis=0), 0).astype(int)
    # every tile gets at least one chunk so the PSUM accumulate chain exists
    kA = np.maximum(kA, 1)

    dst_rel_s = (dst[order] % P).astype(np.float32)

    def build_group(c, g, kX):
        """Concatenate this core's per-tile edge lists for group g, padding
        each tile to kX[t]*128 edges with (idx=0, dst_rel=-1)."""
        idx_parts = []
        rel_parts = []
        for t in range(cfg.TPC):
            key_base = (c * cfg.TPC + t) * 2 + g
            lo, hi = bounds[key_base], bounds[key_base + 1]
            n_real = hi - lo
            n_slots = kX[t] * P
            assert n_real <= n_slots
            idx = np.zeros(n_slots, dtype=np.int16)
            rel = np.full(n_slots, -1.0, dtype=np.float32)
            if n_real:
                s = src_s[lo:hi]
                idx[:n_real] = (s - (cfg.SPLIT if g else 0)).astype(np.int16)
                rel[:n_real] = dst_rel_s[lo:hi]
            idx_parts.append(idx)
            rel_parts.append(rel)
        return np.concatenate(idx_parts), np.concatenate(rel_parts)

    def idx_layout(flat16):
        # dma_gather index layout: [128, n/16]; idx i at (i%16, i//16),
        # replicated across the 8 Q7 cores (partitions 16k+r == r).
        cols = flat16.reshape(-1, 16).T          # [16, cols]
        return np.ascontiguousarray(np.tile(cols, (8, 1)))  # [128, cols]

    def rel_layout(flat):
        return np.ascontiguousarray(flat.reshape(-1, P).T)  # [128, n_chunks]

    x_pad = np.zeros((cfg.NPAD, cfg.IN), dtype=np.float32)
    x_pad[:N] = np.asarray(x, dtype=np.float32)
    x_tab = pack_hilo(x_pad, P)                  # [NPAD, 128] bf16 hi|lo

    per_core = []
    for c in range(cfg.NCORES):
        idxA, relA = build_group(c, 0, kA)
        idxB, relB = build_group(c, 1, kB)
        xT_own = np.ascontiguousarray(
            x_pad[c * cfg.NPC:(c + 1) * cfg.NPC].T)   # [IN, NPC]
        per_core.append(dict(
            x_tab=x_tab,
            xT_own=xT_own,
            idxA=idx_layout(idxA),
            idxB=idx_layout(idxB),
            drA=rel_layout(relA),
            drB=rel_layout(relB),
        ))
    return list(kA), list(kB), per_core


def make_weight_inputs(W1_rel, b1, W1_root, W2_rel, b2, W2_root, cfg):
    f = np.float32
    w2relT = np.asarray(W2_rel, f).T              # [HID, OUT]
    w2rootT = np.asarray(W2_root, f).T            # [HID, OUT]
    ident = np.eye(P, dtype=f)
    return dict(
        w1relT=np.ascontiguousarray(np.asarray(W1_rel, f).T),    # [IN, HID]
        w1rootT=np.ascontiguousarray(np.asarray(W1_root, f).T),  # [IN, HID]
        b1=np.asarray(b1, f).reshape(cfg.HID, 1).copy(),
        w2bothT=np.ascontiguousarray(
            np.concatenate([w2relT, w2rootT], axis=1)),          # [HID, 2*OUT]
        b2rep=np.tile(np.asarray(b2, f).reshape(1, cfg.OUT), (P, 1)).copy(),
        iota=np.tile(np.arange(P, dtype=f).reshape(1, P), (P, 1)).copy(),
        ident=ident,
    )


# ---------------------------------------------------------------- bass build

def build(cfg, kA, kB):
    import concourse.bacc as bacc
    import concourse.tile as tile
    from concourse import mybir

    f32 = mybir.dt.float32
    bf16 = mybir.dt.bfloat16
    i16 = mybir.dt.int16
    Alu = mybir.AluOpType
    Act = mybir.ActivationFunctionType

    IN, HID, OUT, TPC = cfg.IN, cfg.HID, cfg.OUT, cfg.TPC
    NAc, NBc = sum(kA), sum(kB)
    offA = np.concatenate([[0], np.cumsum(kA)]).astype(int)
    offB = np.concatenate([[0], np.cumsum(kB)]).astype(int)

    nc = bacc.Bacc("TRN2", target_bir_lowering=False, debug=False,
                   num_devices=cfg.NCORES, num_swdge_queues=4)

    x_tab = nc.dram_tensor("x_tab", [cfg.NPAD, P], bf16, kind="ExternalInput")
    xT_own_d = nc.dram_tensor("xT_own", [IN, cfg.NPC], f32, kind="ExternalInput")
    idxA_d = nc.dram_tensor("idxA", [P, NAc * 8], i16, kind="ExternalInput")
    idxB_d = nc.dram_tensor("idxB", [P, NBc * 8], i16, kind="ExternalInput")
    drA_d = nc.dram_tensor("drA", [P, NAc], f32, kind="ExternalInput")
    drB_d = nc.dram_tensor("drB", [P, NBc], f32, kind="ExternalInput")
    w1relT_d = nc.dram_tensor("w1relT", [IN, HID], f32, kind="ExternalInput")
    w1rootT_d = nc.dram_tensor("w1rootT", [IN, HID], f32, kind="ExternalInput")
    b1_d = nc.dram_tensor("b1", [HID, 1], f32, kind="ExternalInput")
    w2bothT_d = nc.dram_tensor("w2bothT", [HID, 2 * OUT], f32,
                               kind="ExternalInput")
    b2rep_d = nc.dram_tensor("b2rep", [P, OUT], f32, kind="ExternalInput")
    iota_d = nc.dram_tensor("iota", [P, P], f32, kind="ExternalInput")
    ident_d = nc.dram_tensor("ident", [P, P], f32, kind="ExternalInput")
    out_d = nc.dram_tensor("out", [cfg.NPC, OUT], f32, kind="ExternalOutput")

    segs = [(s, min(s + cfg.SEG, TPC)) for s in range(0, TPC, cfg.SEG)]

    with tile.TileContext(nc) as tc:
        with (
            tc.tile_pool(name="const", bufs=1) as cp,
            tc.tile_pool(name="dram", bufs=1, space="DRAM") as dp,
        ):
            # ---- resident constants
            iota_s = cp.tile([P, P], f32)
            nc.sync.dma_start(iota_s[:], iota_d[:])
            ident_s = cp.tile([P, P], f32)
            nc.sync.dma_start(ident_s[:], ident_d[:])
            w1relT_s = cp.tile([IN, HID], f32)
            nc.sync.dma_start(w1relT_s[:], w1relT_d[:])
            w1rootT_s = cp.tile([IN, HID], f32)
            nc.sync.dma_start(w1rootT_s[:], w1rootT_d[:])
            b1_s = cp.tile([HID, 1], f32)
            nc.sync.dma_start(b1_s[:], b1_d[:])
            w2bothT_s = cp.tile([HID, 2 * OUT], f32)
            nc.sync.dma_start(w2bothT_s[:], w2bothT_d[:])
            b2_s = cp.tile([P, OUT], f32)
            nc.sync.dma_start(b2_s[:], b2rep_d[:])
            xT_own_s = cp.tile([IN, cfg.NPC], f32)
            nc.sync.dma_start(xT_own_s[:], xT_own_d[:])
            idxA_s = cp.tile([P, NAc * 8], i16)
            nc.sync.dma_start(idxA_s[:], idxA_d[:])
            idxB_s = cp.tile([P, NBc * 8], i16)
            nc.sync.dma_start(idxB_s[:], idxB_d[:])
            drA_s = cp.tile([P, NAc], f32)
            nc.sync.dma_start(drA_s[:], drA_d[:])
            drB_s = cp.tile([P, NBc], f32)
            nc.sync.dma_start(drB_s[:], drB_d[:])
            hroots = cp.tile([P, TPC * OUT], f32)

            hp_local = dp.tile([cfg.NPC, P], bf16)
            hp_full = dp.tile([cfg.NPAD, P], bf16)

            maxA = max(offA[t1] - offA[t0] for t0, t1 in segs)
            maxB = max(offB[t1] - offB[t0] for t0, t1 in segs)

            gq = [0]

            def layer(phase, gtabA, gtabB, consume_tile):
                """One gather+segment-sum pass over all tiles."""
                width = P if phase == 1 else 2 * OUT
                with (
                    tc.tile_pool(name=f"G{phase}", bufs=2) as gp,
                    tc.tile_pool(name=f"S{phase}", bufs=4) as sp,
                    tc.tile_pool(name=f"agg{phase}", bufs=2, space="PSUM") as ap,
                ):
                    def gather_piece(G, gtab, idx_s, base, c0, c1):
                        # round-robin the 4 SWDGE queues; each ring caps at
                        # 1024 descriptors and a lone queue stalls the ucode
                        # on ring drain.
                        nc.gpsimd.dma_gather(
                            G[:, c0:c1, :], gtab,
                            idx_s[:, (base + c0) * 8:(base + c1) * 8],
                            (c1 - c0) * P, (c1 - c0) * P, P,
                            queue_num=gq[0] % 4)
                        gq[0] += 1

                    for t0, t1 in segs:
                        a0, a1 = offA[t0], offA[t1]
                        b0, b1_ = offB[t0], offB[t1]
                        nA, nB = a1 - a0, b1_ - b0
                        # dma_gather fails above 1024 indices per call
                        # (HW ring limit) -- split into <=8-chunk pieces.
                        GMAX = 8
                        GA = gp.tile([P, maxA, P], bf16, tag="GA")
                        for c0 in range(0, nA, GMAX):
                            gather_piece(GA, gtabA, idxA_s, a0,
                                         c0, min(c0 + GMAX, nA))
                        GB = gp.tile([P, maxB, P], bf16, tag="GB")
                        for c0 in range(0, nB, GMAX):
                            gather_piece(GB, gtabB, idxB_s, b0,
                                         c0, min(c0 + GMAX, nB))
                        kmaxA = max(kA)
                        kmaxB = max(kB) if max(kB) else 1
                        for t in range(t0, t1):
                            nch = kA[t] + kB[t]
                            acc = ap.tile([P, width], f32, tag="acc")
                            ci = 0
                            for g, G, off, soff, dr, kmax in (
                                (0, GA, offA[t] - a0, offA[t], drA_s, kmaxA),
                                (1, GB, offB[t] - b0, offB[t], drB_s, kmaxB),
                            ):
                                kk = kA[t] if g == 0 else kB[t]
                                if kk == 0:
                                    continue
                                S = sp.tile([P, kmax, P], bf16, tag=f"S{g}")
                                nc.vector.tensor_tensor(
                                    out=S[:, :kk, :],
                                    in0=iota_s[:].unsqueeze(1)
                                        .to_broadcast([P, kk, P]),
                                    in1=dr[:, soff:soff + kk].unsqueeze(2)
                                        .to_broadcast([P, kk, P]),
                                    op=Alu.is_equal)
                                for j in range(kk):
                                    # acc[d, hi|lo] += sum_e S[e,d] G[e,:]
                                    # selector one-hot is exact in bf16;
                                    # hi+lo re-add recovers f32 precision.
                                    nc.tensor.matmul(
                                        acc[:], lhsT=S[:, j, :],
                                        rhs=G[:, off + j, :width],
                                        start=(ci == 0),
                                        stop=(ci == nch - 1))
                                    ci += 1
                            consume_tile(t, acc)

            # ---------------- phase 1
            with (
                tc.tile_pool(name="sb1", bufs=3) as sb1,
                tc.tile_pool(name="hps", bufs=2, space="PSUM") as hps,
            ):
                def consume1(t, acc):
                    # agg[d, f] = hi + lo halves (DVE reads at most one PSUM
                    # input: stage hi through scalar first)
                    agg_sb = sb1.tile([P, IN], f32, tag="aggds")
                    nc.scalar.activation(agg_sb[:], acc[:, :IN], Act.Copy)
                    nc.vector.tensor_tensor(
                        out=agg_sb[:], in0=agg_sb[:], in1=acc[:, IN:2 * IN],
                        op=Alu.add)
                    # transpose to [f, d] for the f32 projections
                    aggT_ps = hps.tile([IN, P], f32, tag="aggT")
                    nc.tensor.transpose(aggT_ps[:], agg_sb[:], ident_s[:])
                    aggsb = sb1.tile([IN, P], f32, tag="aggsb")
                    nc.scalar.activation(aggsb[:], aggT_ps[:], Act.Copy)
                    hT_ps = hps.tile([HID, P], f32, tag="hT")
                    nc.tensor.matmul(hT_ps[:], lhsT=w1relT_s[:], rhs=aggsb[:],
                                     start=True, stop=False)
                    nc.tensor.matmul(hT_ps[:], lhsT=w1rootT_s[:],
                                     rhs=xT_own_s[:, t * P:(t + 1) * P],
                                     start=False, stop=True)
                    hT_sb = sb1.tile([HID, P], f32, tag="hTsb")
                    nc.scalar.activation(hT_sb[:], hT_ps[:], Act.Relu,
                                         bias=b1_s[:, 0:1])
                    hh_ps = hps.tile([P, 2 * OUT], f32, tag="hh")
                    nc.tensor.matmul(hh_ps[:], lhsT=hT_sb[:],
                                     rhs=w2bothT_s[:], start=True, stop=True)
                    nc.vector.tensor_tensor(
                        out=hroots[:, t * OUT:(t + 1) * OUT],
                        in0=hh_ps[:, OUT:], in1=b2_s[:], op=Alu.add)
                    # split hp into bf16 hi/lo packed row [hi|lo|garbage]
                    hp_pack = sb1.tile([P, P], bf16, tag="hppack")
                    nc.vector.tensor_copy(out=hp_pack[:, :OUT],
                                          in_=hh_ps[:, :OUT])
                    nc.vector.tensor_tensor(
                        out=hp_pack[:, OUT:2 * OUT], in0=hh_ps[:, :OUT],
                        in1=hp_pack[:, :OUT], op=Alu.subtract)
                    nc.sync.dma_start(
                        out=hp_local[t * P:(t + 1) * P, :],
                        in_=hp_pack[:])

                layer(1, x_tab[:cfg.SPLIT, :], x_tab[cfg.SPLIT:, :], consume1)

            # ---------------- allgather hp
            nc.gpsimd.collective_compute(
                "AllGather", mybir.AluOpType.bypass,
                replica_groups=[list(range(cfg.NCORES))],
                ins=[hp_local[:cfg.NPC, :]],
                outs=[hp_full[:cfg.NPAD, :]],
            )

            # ---------------- phase 2
            with tc.tile_pool(name="sb2", bufs=3) as sb2:
                def consume2(t, acc):
                    o1 = sb2.tile([P, OUT], f32, tag="o1")
                    nc.scalar.activation(o1[:], acc[:, :OUT], Act.Copy)
                    nc.vector.tensor_tensor(
                        out=o1[:], in0=o1[:], in1=acc[:, OUT:2 * OUT],
                        op=Alu.add)
                    nc.vector.tensor_tensor(
                        out=o1[:], in0=o1[:],
                        in1=hroots[:, t * OUT:(t + 1) * OUT], op=Alu.add)
                    mx = sb2.tile([P, 1], f32, tag="mx")
                    nc.vector.reduce_max(out=mx[:], in_=o1[:],
                                         axis=mybir.AxisListType.X)
                    nmx = sb2.tile([P, 1], f32, tag="nmx")
                    nc.vector.tensor_scalar(nmx[:], mx[:], -1.0, None, Alu.mult)
                    esc = sb2.tile([P, OUT], f32, tag="esc")
                    ssum = sb2.tile([P, 1], f32, tag="ssum")
                    nc.scalar.activation(esc[:], o1[:], Act.Exp,
                                         bias=nmx[:, 0:1], accum_out=ssum[:])
                    lse = sb2.tile([P, 1], f32, tag="lse")
                    nc.scalar.activation(lse[:], ssum[:], Act.Ln)
                    shift = sb2.tile([P, 1], f32, tag="shift")
                    nc.vector.tensor_tensor(out=shift[:], in0=mx[:],
                                            in1=lse[:], op=Alu.add)
                    res = sb2.tile([P, OUT], f32, tag="res")
                    nc.vector.tensor_tensor(
                        out=res[:], in0=o1[:],
                        in1=shift[:, 0:1].to_broadcast([P, OUT]),
                        op=Alu.subtract)
                    nc.sync.dma_start(out=out_d[t * P:(t + 1) * P, :],
                                      in_=res[:])

                layer(2, hp_full[:cfg.SPLIT, :], hp_full[cfg.SPLIT:, :],
                      consume2)

    nc.compile()
    return nc


# ---------------------------------------------------------------- runner

_CACHE = {}


def _get_program(cfg, kA, kB):
    key = (cfg.N, cfg.E, cfg.NCORES, cfg.TPC, tuple(kA), tuple(kB))
    if key not in _CACHE:
        _CACHE[key] = build(cfg, kA, kB)
    return _CACHE[key]


def run_gcn(inputs, cfg, trace=False):
    from concourse import bass_utils

    kA, kB, per_core = preprocess(inputs["x"], inputs["edge_index"], cfg)
    wts = make_weight_inputs(inputs["W1_rel"], inputs["b1"], inputs["W1_root"],
                             inputs["W2_rel"], inputs["b2"], inputs["W2_root"],
                             cfg)
    nc = _get_program(cfg, kA, kB)
    in_maps = []
    for c in range(cfg.NCORES):
        m = dict(per_core[c])
        m.update(wts)
        in_maps.append({k: m[k] for k in (
            "x_tab", "xT_own", "idxA", "idxB", "drA", "drB",
            "w1relT", "w1rootT", "b1", "w2bothT", "b2rep", "iota", "ident")})
    res = bass_utils.run_bass_kernel_spmd(
        nc, in_maps, core_ids=list(range(cfg.NCORES)), trace=trace)
    outs = [res.results[c]["out"] for c in range(cfg.NCORES)]
    full = np.concatenate(outs, axis=0)[:cfg.N]
    return full, res


def kernel(**inputs):
    out, _ = run_gcn(inputs, REAL_CFG)
    return out


# revision 23
# speedup vs baseline: 2.6099x; 1.1294x over previous
"""GCN (2-layer GraphConv + ReLU + log_softmax) on 8 Trainium2 NeuronCores.

Strategy (graph/data parallel, per sharding hint):
  - Nodes are padded to 50176 = 8 * 49 * 128 and sharded contiguously:
    core c owns nodes [c*6272, (c+1)*6272).
  - Edges are routed to the core owning their *destination* node, sorted by
    destination tile (128 nodes), split into A/B groups by source index
    (src < 32768 vs >= 32768, so gather indices fit int16), and padded to
    128-edge chunks per (tile, group).
  - Gathered feature rows use a bf16 hi/lo split packed into one 256B row:
    row = [bf16(v) (64/40 cols) | bf16(v - hi) | pad]. The segment-sum
    matmul streams both halves through the PE in a single bf16 op
    (selector one-hot stationary, exact in bf16) and the two halves are
    re-added afterwards -- f32-quality precision (~2^-18 rel) at bf16
    matmul speed, with the same 256B/row gather wire cost as f32.
  - Gathers round-robin the 4 SWDGE queues (a single queue stalls the
    gather ucode on ring drain: 8.3us vs 2.9us per 1024-row call).
  - Layer 1 per core: dma_gather x[src] rows from the replicated packed x
    table in HBM -> per-chunk one-hot bf16 selector (DVE is_equal) ->
    one PE matmul per chunk accumulating aggT[128d, hi|lo] in PSUM ->
    hi+lo add -> PE transpose -> projections (f32) -> ReLU -> hT.
  - The layer-2 "rel" projection is applied *before* the second gather
    (segment_sum is linear): hp = h @ W2_rel.T per tile, split hi/lo bf16,
    packed to [NPC, 128] bf16, AllGathered (1.6MB/rank). Layer-2 root term
    hroot = h @ W2_root.T + b2 stays resident in SBUF.
  - Layer 2 per core: dma_gather hp rows, same selector matmul into
    acc2[128d, 40hi|40lo], hi+lo+hroot adds, log_softmax, DMA out.
"""

import numpy as np

# ---------------------------------------------------------------- problem cfg

P = 128


class Cfg:
    def __init__(self, n, e, in_ch, hid, out_ch, n_cores, tiles_per_core, split,
                 seg_tiles=7):
        self.N = n
        self.E = e
        self.IN = in_ch
        self.HID = hid
        self.OUT = out_ch
        self.NCORES = n_cores
        self.TPC = tiles_per_core
        self.NPC = tiles_per_core * P
        self.NPAD = self.NPC * n_cores
        self.SPLIT = split
        self.SEG = seg_tiles
        assert self.NPAD >= self.N
        assert self.SPLIT % P == 0


REAL_CFG = Cfg(n=50000, e=800000, in_ch=64, hid=128, out_ch=40, n_cores=8,
               tiles_per_core=49, split=32768, seg_tiles=7)


def seg_perm(cfg):
    """Permutation: global node id -> segment-major gather-table row.

    Table rows are ordered [segment][core][tile-in-seg][node-in-tile] so a
    per-segment AllGather (each core contributing its 7-tile slice) writes a
    contiguous block, letting the collective overlap layer-1 compute.
    """
    n = np.arange(cfg.NPAD)
    c, rem = n // cfg.NPC, n % cfg.NPC
    t, r = rem // P, rem % P
    s, ts = t // cfg.SEG, t % cfg.SEG
    segrows = cfg.SEG * P
    return (s * (cfg.NCORES * segrows) + c * segrows + ts * P + r)


# ---------------------------------------------------------------- host preproc

def _bf16_round(x):
    """Round f32 -> bf16 (round-to-nearest-even), keep as float32 bits."""
    u = x.view(np.uint32)
    r = (u + 0x7FFF + ((u >> 16) & 1)) & 0xFFFF0000
    return r.view(np.float32)


def pack_hilo(v, width):
    """[R, C] f32 -> [R, width] bf16 rows [hi(C) | lo(C) | pad]."""
    import ml_dtypes
    hi = _bf16_round(np.ascontiguousarray(v))
    lo = _bf16_round(np.ascontiguousarray(v - hi))
    out = np.zeros((v.shape[0], width), dtype=ml_dtypes.bfloat16)
    out[:, :v.shape[1]] = hi.astype(ml_dtypes.bfloat16)
    out[:, v.shape[1]:2 * v.shape[1]] = lo.astype(ml_dtypes.bfloat16)
    return out


def preprocess(x, edge_index, cfg):
    """Build per-core gather-index / selector tensors and the chunk structure.

    Returns (kA, kB, per_core) where kA/kB are per-tile chunk counts (uniform
    across cores; padded to the max) and per_core is a list of dicts of numpy
    arrays for each core's in_map.
    """
    N, E = cfg.N, cfg.E
    perm = seg_perm(cfg)
    src = perm[np.asarray(edge_index[0], dtype=np.int64)]  # permuted table row
    dst = np.asarray(edge_index[1], dtype=np.int64)

    gtile = dst // P                      # global dst tile in [0, NCORES*TPC)
    grp = (src >= cfg.SPLIT).astype(np.int64)
    key = gtile * 2 + grp
    order = np.argsort(key, kind="stable")
    src_s = src[order]
    key_s = key[order]

    nkeys = cfg.NCORES * cfg.TPC * 2
    bounds = np.searchsorted(key_s, np.arange(nkeys + 1))
    counts = np.diff(bounds).reshape(cfg.NCORES, cfg.TPC, 2)

    kA = np.maximum(np.ceil(counts[:, :, 0] / P).max(axis=0), 0).astype(int)
    kB = np.maximum(np.ceil(counts[:, :, 1] / P).max(axis=0), 0).astype(int)
    # every tile gets at least one chunk so the PSUM accumulate chain exists
    kA = np.maximum(kA, 1)

    dst_rel_s = (dst[order] % P).astype(np.float32)

    def build_group(c, g, kX):
        """Concatenate this core's per-tile edge lists for group g, padding
        each tile to kX[t]*128 edges with (idx=0, dst_rel=-1)."""
        idx_parts = []
        rel_parts = []
        for t in range(cfg.TPC):
            key_base = (c * cfg.TPC + t) * 2 + g
            lo, hi = bounds[key_base], bounds[key_base + 1]
            n_real = hi - lo
            n_slots = kX[t] * P
            assert n_real <= n_slots
            idx = np.zeros(n_slots, dtype=np.int16)
            rel = np.full(n_slots, -1.0, dtype=np.float32)
            if n_real:
                s = src_s[lo:hi]
                idx[:n_real] = (s - (cfg.SPLIT if g else 0)).astype(np.int16)
                rel[:n_real] = dst_rel_s[lo:hi]
            idx_parts.append(idx)
            rel_parts.append(rel)
        return np.concatenate(idx_parts), np.concatenate(rel_parts)

    def idx_layout(flat16):
        # dma_gather index layout: [128, n/16]; idx i at (i%16, i//16),
        # replicated across the 8 Q7 cores (partitions 16k+r == r).
        cols = flat16.reshape(-1, 16).T          # [16, cols]
        return np.ascontiguousarray(np.tile(cols, (8, 1)))  # [128, cols]

    def rel_layout(flat):
        return np.ascontiguousarray(flat.reshape(-1, P).T)  # [128, n_chunks]

    x_pad = np.zeros((cfg.NPAD, cfg.IN), dtype=np.float32)
    x_pad[:N] = np.asarray(x, dtype=np.float32)
    # gather table in segment-major permuted row order: row perm[n] = x[n]
    x_tab = pack_hilo(x_pad[np.argsort(perm)], P)  # [NPAD, 128] bf16 hi|lo

    per_core = []
    for c in range(cfg.NCORES):
        idxA, relA = build_group(c, 0, kA)
        idxB, relB = build_group(c, 1, kB)
        xT_own = np.ascontiguousarray(
            x_pad[c * cfg.NPC:(c + 1) * cfg.NPC].T)   # [IN, NPC]
        per_core.append(dict(
            x_tab=x_tab,
            xT_own=xT_own,
            idxA=idx_layout(idxA),
            idxB=idx_layout(idxB),
            drA=rel_layout(relA),
            drB=rel_layout(relB),
        ))
    return list(kA), list(kB), per_core


def make_weight_inputs(W1_rel, b1, W1_root, W2_rel, b2, W2_root, cfg):
    f = np.float32
    w2relT = np.asarray(W2_rel, f).T              # [HID, OUT]
    w2rootT = np.asarray(W2_root, f).T            # [HID, OUT]
    ident = np.eye(P, dtype=f)
    return dict(
        w1relT=np.ascontiguousarray(np.asarray(W1_rel, f).T),    # [IN, HID]
        w1rootT=np.ascontiguousarray(np.asarray(W1_root, f).T),  # [IN, HID]
        b1=np.asarray(b1, f).reshape(cfg.HID, 1).copy(),
        w2bothT=np.ascontiguousarray(
            np.concatenate([w2relT, w2rootT], axis=1)),          # [HID, 2*OUT]
        b2rep=np.tile(np.asarray(b2, f).reshape(1, cfg.OUT), (P, 1)).copy(),
        iota=np.tile(np.arange(P, dtype=f).reshape(1, P), (P, 1)).copy(),
        ident=ident,
    )


# ---------------------------------------------------------------- bass build

def build(cfg, kA, kB):
    import concourse.bacc as bacc
    import concourse.tile as tile
    from concourse import mybir

    f32 = mybir.dt.float32
    bf16 = mybir.dt.bfloat16
    i16 = mybir.dt.int16
    Alu = mybir.AluOpType
    Act = mybir.ActivationFunctionType

    IN, HID, OUT, TPC = cfg.IN, cfg.HID, cfg.OUT, cfg.TPC
    NAc, NBc = sum(kA), sum(kB)
    offA = np.concatenate([[0], np.cumsum(kA)]).astype(int)
    offB = np.concatenate([[0], np.cumsum(kB)]).astype(int)

    nc = bacc.Bacc("TRN2", target_bir_lowering=False, debug=False,
                   num_devices=cfg.NCORES, num_swdge_queues=4)

    x_tab = nc.dram_tensor("x_tab", [cfg.NPAD, P], bf16, kind="ExternalInput")
    xT_own_d = nc.dram_tensor("xT_own", [IN, cfg.NPC], f32, kind="ExternalInput")
    idxA_d = nc.dram_tensor("idxA", [P, NAc * 8], i16, kind="ExternalInput")
    idxB_d = nc.dram_tensor("idxB", [P, NBc * 8], i16, kind="ExternalInput")
    drA_d = nc.dram_tensor("drA", [P, NAc], f32, kind="ExternalInput")
    drB_d = nc.dram_tensor("drB", [P, NBc], f32, kind="ExternalInput")
    w1relT_d = nc.dram_tensor("w1relT", [IN, HID], f32, kind="ExternalInput")
    w1rootT_d = nc.dram_tensor("w1rootT", [IN, HID], f32, kind="ExternalInput")
    b1_d = nc.dram_tensor("b1", [HID, 1], f32, kind="ExternalInput")
    w2bothT_d = nc.dram_tensor("w2bothT", [HID, 2 * OUT], f32,
                               kind="ExternalInput")
    b2rep_d = nc.dram_tensor("b2rep", [P, OUT], f32, kind="ExternalInput")
    iota_d = nc.dram_tensor("iota", [P, P], f32, kind="ExternalInput")
    ident_d = nc.dram_tensor("ident", [P, P], f32, kind="ExternalInput")
    out_d = nc.dram_tensor("out", [cfg.NPC, OUT], f32, kind="ExternalOutput")

    segs = [(s, min(s + cfg.SEG, TPC)) for s in range(0, TPC, cfg.SEG)]

    with tile.TileContext(nc) as tc:
        with (
            tc.tile_pool(name="const", bufs=1) as cp,
            tc.tile_pool(name="dram", bufs=1, space="DRAM") as dp,
        ):
            # ---- resident constants
            iota_s = cp.tile([P, P], f32)
            nc.sync.dma_start(iota_s[:], iota_d[:])
            ident_s = cp.tile([P, P], f32)
            nc.sync.dma_start(ident_s[:], ident_d[:])
            w1relT_s = cp.tile([IN, HID], f32)
            nc.sync.dma_start(w1relT_s[:], w1relT_d[:])
            w1rootT_s = cp.tile([IN, HID], f32)
            nc.sync.dma_start(w1rootT_s[:], w1rootT_d[:])
            b1_s = cp.tile([HID, 1], f32)
            nc.sync.dma_start(b1_s[:], b1_d[:])
            w2bothT_s = cp.tile([HID, 2 * OUT], f32)
            nc.sync.dma_start(w2bothT_s[:], w2bothT_d[:])
            b2_s = cp.tile([P, OUT], f32)
            nc.sync.dma_start(b2_s[:], b2rep_d[:])
            xT_own_s = cp.tile([IN, cfg.NPC], f32)
            nc.sync.dma_start(xT_own_s[:], xT_own_d[:])
            idxA_s = cp.tile([P, NAc * 8], i16)
            nc.sync.dma_start(idxA_s[:], idxA_d[:])
            idxB_s = cp.tile([P, NBc * 8], i16)
            nc.sync.dma_start(idxB_s[:], idxB_d[:])
            drA_s = cp.tile([P, NAc], f32)
            nc.sync.dma_start(drA_s[:], drA_d[:])
            drB_s = cp.tile([P, NBc], f32)
            nc.sync.dma_start(drB_s[:], drB_d[:])
            hroots = cp.tile([P, TPC * OUT], f32)

            hp_local = dp.tile([cfg.NPC, P], bf16)
            hp_full = dp.tile([cfg.NPAD, P], bf16)

            maxA = max(offA[t1] - offA[t0] for t0, t1 in segs)
            maxB = max(offB[t1] - offB[t0] for t0, t1 in segs)

            gq = [0]

            def layer(phase, gtabA, gtabB, consume_tile):
                """One gather+segment-sum pass over all tiles."""
                width = P if phase == 1 else 2 * OUT
                with (
                    tc.tile_pool(name=f"G{phase}", bufs=2) as gp,
                    tc.tile_pool(name=f"S{phase}", bufs=4) as sp,
                    tc.tile_pool(name=f"agg{phase}", bufs=2, space="PSUM") as ap,
                ):
                    def gather_piece(G, gtab, idx_s, base, c0, c1):
                        # round-robin the 4 SWDGE queues; each ring caps at
                        # 1024 descriptors and a lone queue stalls the ucode
                        # on ring drain.
                        nc.gpsimd.dma_gather(
                            G[:, c0:c1, :], gtab,
                            idx_s[:, (base + c0) * 8:(base + c1) * 8],
                            (c1 - c0) * P, (c1 - c0) * P, P,
                            queue_num=gq[0] % 4)
                        gq[0] += 1

                    for t0, t1 in segs:
                        a0, a1 = offA[t0], offA[t1]
                        b0, b1_ = offB[t0], offB[t1]
                        nA, nB = a1 - a0, b1_ - b0
                        # dma_gather fails above 1024 indices per call
                        # (HW ring limit) -- split into <=8-chunk pieces.
                        GMAX = 8
                        GA = gp.tile([P, maxA, P], bf16, tag="GA")
                        for c0 in range(0, nA, GMAX):
                            gather_piece(GA, gtabA, idxA_s, a0,
                                         c0, min(c0 + GMAX, nA))
                        GB = gp.tile([P, maxB, P], bf16, tag="GB")
                        for c0 in range(0, nB, GMAX):
                            gather_piece(GB, gtabB, idxB_s, b0,
                                         c0, min(c0 + GMAX, nB))
                        kmaxA = max(kA)
                        kmaxB = max(kB) if max(kB) else 1
                        for t in range(t0, t1):
                            nch = kA[t] + kB[t]
                            acc = ap.tile([P, width], f32, tag="acc")
                            ci = 0
                            for g, G, off, soff, dr, kmax in (
                                (0, GA, offA[t] - a0, offA[t], drA_s, kmaxA),
                                (1, GB, offB[t] - b0, offB[t], drB_s, kmaxB),
                            ):
                                kk = kA[t] if g == 0 else kB[t]
                                if kk == 0:
                                    continue
                                S = sp.tile([P, kmax, P], bf16, tag=f"S{g}")
                                nc.vector.tensor_tensor(
                                    out=S[:, :kk, :],
                                    in0=iota_s[:].unsqueeze(1)
                                        .to_broadcast([P, kk, P]),
                                    in1=dr[:, soff:soff + kk].unsqueeze(2)
                                        .to_broadcast([P, kk, P]),
                                    op=Alu.is_equal)
                                for j in range(kk):
                                    # acc[d, hi|lo] += sum_e S[e,d] G[e,:]
                                    # selector one-hot is exact in bf16;
                                    # hi+lo re-add recovers f32 precision.
                                    nc.tensor.matmul(
                                        acc[:], lhsT=S[:, j, :],
                                        rhs=G[:, off + j, :width],
                                        start=(ci == 0),
                                        stop=(ci == nch - 1))
                                    ci += 1
                            consume_tile(t, acc)

            # ---------------- phase 1
            with (
                tc.tile_pool(name="sb1", bufs=3) as sb1,
                tc.tile_pool(name="hps", bufs=2, space="PSUM") as hps,
            ):
                def consume1(t, acc):
                    # agg[d, f] = hi + lo halves (DVE reads at most one PSUM
                    # input: stage hi through scalar first)
                    agg_sb = sb1.tile([P, IN], f32, tag="aggds")
                    nc.scalar.activation(agg_sb[:], acc[:, :IN], Act.Copy)
                    nc.vector.tensor_tensor(
                        out=agg_sb[:], in0=agg_sb[:], in1=acc[:, IN:2 * IN],
                        op=Alu.add)
                    # transpose to [f, d] for the f32 projections
                    aggT_ps = hps.tile([IN, P], f32, tag="aggT")
                    nc.tensor.transpose(aggT_ps[:], agg_sb[:], ident_s[:])
                    aggsb = sb1.tile([IN, P], f32, tag="aggsb")
                    nc.scalar.activation(aggsb[:], aggT_ps[:], Act.Copy)
                    hT_ps = hps.tile([HID, P], f32, tag="hT")
                    nc.tensor.matmul(hT_ps[:], lhsT=w1relT_s[:], rhs=aggsb[:],
                                     start=True, stop=False)
                    nc.tensor.matmul(hT_ps[:], lhsT=w1rootT_s[:],
                                     rhs=xT_own_s[:, t * P:(t + 1) * P],
                                     start=False, stop=True)
                    hT_sb = sb1.tile([HID, P], f32, tag="hTsb")
                    nc.scalar.activation(hT_sb[:], hT_ps[:], Act.Relu,
                                         bias=b1_s[:, 0:1])
                    hh_ps = hps.tile([P, 2 * OUT], f32, tag="hh")
                    nc.tensor.matmul(hh_ps[:], lhsT=hT_sb[:],
                                     rhs=w2bothT_s[:], start=True, stop=True)
                    nc.vector.tensor_tensor(
                        out=hroots[:, t * OUT:(t + 1) * OUT],
                        in0=hh_ps[:, OUT:], in1=b2_s[:], op=Alu.add)
                    # split hp into bf16 hi/lo packed row [hi|lo|garbage]
                    hp_pack = sb1.tile([P, P], bf16, tag="hppack")
                    nc.vector.tensor_copy(out=hp_pack[:, :OUT],
                                          in_=hh_ps[:, :OUT])
                    nc.vector.tensor_tensor(
                        out=hp_pack[:, OUT:2 * OUT], in0=hh_ps[:, :OUT],
                        in1=hp_pack[:, :OUT], op=Alu.subtract)
                    nc.sync.dma_start(
                        out=hp_local[t * P:(t + 1) * P, :],
                        in_=hp_pack[:])

                # Per-segment AllGather chunks fire as soon as each 7-tile
                # slice of hp_local is written, overlapping layer-1 compute.
                # hp_full rows are segment-major (seg_perm) so each chunk's
                # 8-rank output block is contiguous.
                segrows = cfg.SEG * P

                def allgather_seg(si):
                    nc.gpsimd.collective_compute(
                        "AllGather", mybir.AluOpType.bypass,
                        replica_groups=[list(range(cfg.NCORES))],
                        ins=[hp_local[si * segrows:(si + 1) * segrows, :]],
                        outs=[hp_full[si * segrows * cfg.NCORES:
                                      (si + 1) * segrows * cfg.NCORES, :]],
                    )

                nseg = len(segs)
                done_segs = [0]

                def consume1_and_gather(t, acc):
                    consume1(t, acc)
                    si = done_segs[0]
                    if t == segs[si][1] - 1:
                        allgather_seg(si)
                        done_segs[0] += 1

                layer(1, x_tab[:cfg.SPLIT, :], x_tab[cfg.SPLIT:, :],
                      consume1_and_gather)
                assert done_segs[0] == nseg

            # ---------------- phase 2
            with tc.tile_pool(name="sb2", bufs=3) as sb2:
                def consume2(t, acc):
                    # two adds, each with at most one PSUM operand
                    o1 = sb2.tile([P, OUT], f32, tag="o1")
                    nc.vector.tensor_tensor(
                        out=o1[:], in0=acc[:, :OUT],
                        in1=hroots[:, t * OUT:(t + 1) * OUT], op=Alu.add)
                    nc.vector.tensor_tensor(
                        out=o1[:], in0=o1[:], in1=acc[:, OUT:2 * OUT],
                        op=Alu.add)
                    mx = sb2.tile([P, 1], f32, tag="mx")
                    nc.vector.reduce_max(out=mx[:], in_=o1[:],
                                         axis=mybir.AxisListType.X)
                    nmx = sb2.tile([P, 1], f32, tag="nmx")
                    nc.vector.tensor_scalar(nmx[:], mx[:], -1.0, None, Alu.mult)
                    esc = sb2.tile([P, OUT], f32, tag="esc")
                    ssum = sb2.tile([P, 1], f32, tag="ssum")
                    nc.scalar.activation(esc[:], o1[:], Act.Exp,
                                         bias=nmx[:, 0:1], accum_out=ssum[:])
                    lse = sb2.tile([P, 1], f32, tag="lse")
                    nc.scalar.activation(lse[:], ssum[:], Act.Ln)
                    shift = sb2.tile([P, 1], f32, tag="shift")
                    nc.vector.tensor_tensor(out=shift[:], in0=mx[:],
                                            in1=lse[:], op=Alu.add)
                    res = sb2.tile([P, OUT], f32, tag="res")
                    nc.vector.tensor_tensor(
                        out=res[:], in0=o1[:],
                        in1=shift[:, 0:1].to_broadcast([P, OUT]),
                        op=Alu.subtract)
                    nc.sync.dma_start(out=out_d[t * P:(t + 1) * P, :],
                                      in_=res[:])

                layer(2, hp_full[:cfg.SPLIT, :], hp_full[cfg.SPLIT:, :],
                      consume2)

    nc.compile()
    return nc


# ---------------------------------------------------------------- runner

_CACHE = {}


def _get_program(cfg, kA, kB):
    key = (cfg.N, cfg.E, cfg.NCORES, cfg.TPC, tuple(kA), tuple(kB))
    if key not in _CACHE:
        _CACHE[key] = build(cfg, kA, kB)
    return _CACHE[key]


def run_gcn(inputs, cfg, trace=False):
    from concourse import bass_utils

    kA, kB, per_core = preprocess(inputs["x"], inputs["edge_index"], cfg)
    wts = make_weight_inputs(inputs["W1_rel"], inputs["b1"], inputs["W1_root"],
                             inputs["W2_rel"], inputs["b2"], inputs["W2_root"],
                             cfg)
    nc = _get_program(cfg, kA, kB)
    in_maps = []
    for c in range(cfg.NCORES):
        m = dict(per_core[c])
        m.update(wts)
        in_maps.append({k: m[k] for k in (
            "x_tab", "xT_own", "idxA", "idxB", "drA", "drB",
            "w1relT", "w1rootT", "b1", "w2bothT", "b2rep", "iota", "ident")})
    res = bass_utils.run_bass_kernel_spmd(
        nc, in_maps, core_ids=list(range(cfg.NCORES)), trace=trace)
    outs = [res.results[c]["out"] for c in range(cfg.NCORES)]
    full = np.concatenate(outs, axis=0)[:cfg.N]
    return full, res


def kernel(**inputs):
    out, _ = run_gcn(inputs, REAL_CFG)
    return out


# revision 25
# speedup vs baseline: 2.6918x; 1.0314x over previous
"""GCN (2-layer GraphConv + ReLU + log_softmax) on 8 Trainium2 NeuronCores.

Strategy (graph/data parallel, per sharding hint):
  - Nodes are padded to 50176 = 8 * 49 * 128 and sharded contiguously:
    core c owns nodes [c*6272, (c+1)*6272).
  - Edges are routed to the core owning their *destination* node, sorted by
    destination tile (128 nodes), split into A/B groups by source index
    (src < 32768 vs >= 32768, so gather indices fit int16), and padded to
    128-edge chunks per (tile, group).
  - Gathered feature rows use a bf16 hi/lo split packed into one 256B row:
    row = [bf16(v) (64/40 cols) | bf16(v - hi) | pad]. The segment-sum
    matmul streams both halves through the PE in a single bf16 op
    (selector one-hot stationary, exact in bf16) and the two halves are
    re-added afterwards -- f32-quality precision (~2^-18 rel) at bf16
    matmul speed, with the same 256B/row gather wire cost as f32.
  - Gathers round-robin the 4 SWDGE queues (a single queue stalls the
    gather ucode on ring drain: 8.3us vs 2.9us per 1024-row call).
  - Layer 1 per core: dma_gather x[src] rows from the replicated packed x
    table in HBM -> per-chunk one-hot bf16 selector (DVE is_equal) ->
    one PE matmul per chunk accumulating aggT[128d, hi|lo] in PSUM ->
    hi+lo add -> PE transpose -> projections (f32) -> ReLU -> hT.
  - The layer-2 "rel" projection is applied *before* the second gather
    (segment_sum is linear): hp = h @ W2_rel.T per tile, split hi/lo bf16,
    packed to [NPC, 128] bf16, AllGathered (1.6MB/rank). Layer-2 root term
    hroot = h @ W2_root.T + b2 stays resident in SBUF.
  - Layer 2 per core: dma_gather hp rows, same selector matmul into
    acc2[128d, 40hi|40lo], hi+lo+hroot adds, log_softmax, DMA out.
"""

import numpy as np

# ---------------------------------------------------------------- problem cfg

P = 128


class Cfg:
    def __init__(self, n, e, in_ch, hid, out_ch, n_cores, tiles_per_core, split,
                 seg_tiles=7):
        self.N = n
        self.E = e
        self.IN = in_ch
        self.HID = hid
        self.OUT = out_ch
        self.NCORES = n_cores
        self.TPC = tiles_per_core
        self.NPC = tiles_per_core * P
        self.NPAD = self.NPC * n_cores
        self.SPLIT = split
        self.SEG = seg_tiles
        assert self.NPAD >= self.N
        assert self.SPLIT % P == 0


# SPLIT sits on a segment boundary (4 segs * 8 cores * 896 rows = 28672) so
# the A-table collective covers exactly segs 0-3 and fires before layer 1
# finishes; it must also stay <= 32768 so A-group gather indices fit int16.
REAL_CFG = Cfg(n=50000, e=800000, in_ch=64, hid=128, out_ch=40, n_cores=8,
               tiles_per_core=49, split=28672, seg_tiles=7)


def seg_perm(cfg):
    """Permutation: global node id -> segment-major gather-table row.

    Table rows are ordered [segment][core][tile-in-seg][node-in-tile] so a
    per-segment AllGather (each core contributing its 7-tile slice) writes a
    contiguous block, letting the collective overlap layer-1 compute.
    """
    n = np.arange(cfg.NPAD)
    c, rem = n // cfg.NPC, n % cfg.NPC
    t, r = rem // P, rem % P
    s, ts = t // cfg.SEG, t % cfg.SEG
    segrows = cfg.SEG * P
    return (s * (cfg.NCORES * segrows) + c * segrows + ts * P + r)


# ---------------------------------------------------------------- host preproc

def _bf16_round(x):
    """Round f32 -> bf16 (round-to-nearest-even), keep as float32 bits."""
    u = x.view(np.uint32)
    r = (u + 0x7FFF + ((u >> 16) & 1)) & 0xFFFF0000
    return r.view(np.float32)


def pack_hilo(v, width):
    """[R, C] f32 -> [R, width] bf16 rows [hi(C) | lo(C) | pad]."""
    import ml_dtypes
    hi = _bf16_round(np.ascontiguousarray(v))
    lo = _bf16_round(np.ascontiguousarray(v - hi))
    out = np.zeros((v.shape[0], width), dtype=ml_dtypes.bfloat16)
    out[:, :v.shape[1]] = hi.astype(ml_dtypes.bfloat16)
    out[:, v.shape[1]:2 * v.shape[1]] = lo.astype(ml_dtypes.bfloat16)
    return out


def preprocess(x, edge_index, cfg):
    """Build per-core gather-index / selector tensors and the chunk structure.

    Returns (kA, kB, per_core) where kA/kB are per-tile chunk counts (uniform
    across cores; padded to the max) and per_core is a list of dicts of numpy
    arrays for each core's in_map.
    """
    N, E = cfg.N, cfg.E
    perm = seg_perm(cfg)
    src = perm[np.asarray(edge_index[0], dtype=np.int64)]  # permuted table row
    dst = np.asarray(edge_index[1], dtype=np.int64)

    gtile = dst // P                      # global dst tile in [0, NCORES*TPC)
    grp = (src >= cfg.SPLIT).astype(np.int64)
    key = gtile * 2 + grp
    order = np.argsort(key, kind="stable")
    src_s = src[order]
    key_s = key[order]

    nkeys = cfg.NCORES * cfg.TPC * 2
    bounds = np.searchsorted(key_s, np.arange(nkeys + 1))
    counts = np.diff(bounds).reshape(cfg.NCORES, cfg.TPC, 2)

    kA = np.maximum(np.ceil(counts[:, :, 0] / P).max(axis=0), 0).astype(int)
    kB = np.maximum(np.ceil(counts[:, :, 1] / P).max(axis=0), 0).astype(int)
    # every tile gets at least one chunk so the PSUM accumulate chain exists
    kA = np.maximum(kA, 1)

    dst_rel_s = (dst[order] % P).astype(np.float32)

    def build_group(c, g, kX):
        """Concatenate this core's per-tile edge lists for group g, padding
        each tile to kX[t]*128 edges with (idx=0, dst_rel=-1)."""
        idx_parts = []
        rel_parts = []
        for t in range(cfg.TPC):
            key_base = (c * cfg.TPC + t) * 2 + g
            lo, hi = bounds[key_base], bounds[key_base + 1]
            n_real = hi - lo
            n_slots = kX[t] * P
            assert n_real <= n_slots
            idx = np.zeros(n_slots, dtype=np.int16)
            rel = np.full(n_slots, -1.0, dtype=np.float32)
            if n_real:
                s = src_s[lo:hi]
                idx[:n_real] = (s - (cfg.SPLIT if g else 0)).astype(np.int16)
                rel[:n_real] = dst_rel_s[lo:hi]
            idx_parts.append(idx)
            rel_parts.append(rel)
        return np.concatenate(idx_parts), np.concatenate(rel_parts)

    def idx_layout(flat16):
        # dma_gather index layout: [128, n/16]; idx i at (i%16, i//16),
        # replicated across the 8 Q7 cores (partitions 16k+r == r).
        cols = flat16.reshape(-1, 16).T          # [16, cols]
        return np.ascontiguousarray(np.tile(cols, (8, 1)))  # [128, cols]

    def rel_layout(flat):
        return np.ascontiguousarray(flat.reshape(-1, P).T)  # [128, n_chunks]

    x_pad = np.zeros((cfg.NPAD, cfg.IN), dtype=np.float32)
    x_pad[:N] = np.asarray(x, dtype=np.float32)
    # gather table in segment-major permuted row order: row perm[n] = x[n]
    x_tab = pack_hilo(x_pad[np.argsort(perm)], P)  # [NPAD, 128] bf16 hi|lo

    per_core = []
    for c in range(cfg.NCORES):
        idxA, relA = build_group(c, 0, kA)
        idxB, relB = build_group(c, 1, kB)
        xT_own = np.ascontiguousarray(
            x_pad[c * cfg.NPC:(c + 1) * cfg.NPC].T)   # [IN, NPC]
        per_core.append(dict(
            x_tab=x_tab,
            xT_own=xT_own,
            idxA=idx_layout(idxA),
            idxB=idx_layout(idxB),
            drA=rel_layout(relA),
            drB=rel_layout(relB),
        ))
    return list(kA), list(kB), per_core


def make_weight_inputs(W1_rel, b1, W1_root, W2_rel, b2, W2_root, cfg):
    f = np.float32
    w2relT = np.asarray(W2_rel, f).T              # [HID, OUT]
    w2rootT = np.asarray(W2_root, f).T            # [HID, OUT]
    ident = np.eye(P, dtype=f)
    return dict(
        w1relT=np.ascontiguousarray(np.asarray(W1_rel, f).T),    # [IN, HID]
        w1rootT=np.ascontiguousarray(np.asarray(W1_root, f).T),  # [IN, HID]
        b1=np.asarray(b1, f).reshape(cfg.HID, 1).copy(),
        w2bothT=np.ascontiguousarray(
            np.concatenate([w2relT, w2rootT], axis=1)),          # [HID, 2*OUT]
        b2rep=np.tile(np.asarray(b2, f).reshape(1, cfg.OUT), (P, 1)).copy(),
        iota=np.tile(np.arange(P, dtype=f).reshape(1, P), (P, 1)).copy(),
        ident=ident,
    )


# ---------------------------------------------------------------- bass build

def build(cfg, kA, kB):
    import concourse.bacc as bacc
    import concourse.tile as tile
    from concourse import mybir

    f32 = mybir.dt.float32
    bf16 = mybir.dt.bfloat16
    i16 = mybir.dt.int16
    Alu = mybir.AluOpType
    Act = mybir.ActivationFunctionType

    IN, HID, OUT, TPC = cfg.IN, cfg.HID, cfg.OUT, cfg.TPC
    NAc, NBc = sum(kA), sum(kB)
    offA = np.concatenate([[0], np.cumsum(kA)]).astype(int)
    offB = np.concatenate([[0], np.cumsum(kB)]).astype(int)

    nc = bacc.Bacc("TRN2", target_bir_lowering=False, debug=False,
                   num_devices=cfg.NCORES, num_swdge_queues=4)

    x_tab = nc.dram_tensor("x_tab", [cfg.NPAD, P], bf16, kind="ExternalInput")
    xT_own_d = nc.dram_tensor("xT_own", [IN, cfg.NPC], f32, kind="ExternalInput")
    idxA_d = nc.dram_tensor("idxA", [P, NAc * 8], i16, kind="ExternalInput")
    idxB_d = nc.dram_tensor("idxB", [P, NBc * 8], i16, kind="ExternalInput")
    drA_d = nc.dram_tensor("drA", [P, NAc], f32, kind="ExternalInput")
    drB_d = nc.dram_tensor("drB", [P, NBc], f32, kind="ExternalInput")
    w1relT_d = nc.dram_tensor("w1relT", [IN, HID], f32, kind="ExternalInput")
    w1rootT_d = nc.dram_tensor("w1rootT", [IN, HID], f32, kind="ExternalInput")
    b1_d = nc.dram_tensor("b1", [HID, 1], f32, kind="ExternalInput")
    w2bothT_d = nc.dram_tensor("w2bothT", [HID, 2 * OUT], f32,
                               kind="ExternalInput")
    b2rep_d = nc.dram_tensor("b2rep", [P, OUT], f32, kind="ExternalInput")
    iota_d = nc.dram_tensor("iota", [P, P], f32, kind="ExternalInput")
    ident_d = nc.dram_tensor("ident", [P, P], f32, kind="ExternalInput")
    out_d = nc.dram_tensor("out", [cfg.NPC, OUT], f32, kind="ExternalOutput")

    segs = [(s, min(s + cfg.SEG, TPC)) for s in range(0, TPC, cfg.SEG)]

    with tile.TileContext(nc) as tc:
        with (
            tc.tile_pool(name="const", bufs=1) as cp,
            tc.tile_pool(name="dram", bufs=1, space="DRAM") as dp,
        ):
            # ---- resident constants
            iota_s = cp.tile([P, P], f32)
            nc.sync.dma_start(iota_s[:], iota_d[:])
            ident_s = cp.tile([P, P], f32)
            nc.sync.dma_start(ident_s[:], ident_d[:])
            w1relT_s = cp.tile([IN, HID], f32)
            nc.sync.dma_start(w1relT_s[:], w1relT_d[:])
            w1rootT_s = cp.tile([IN, HID], f32)
            nc.sync.dma_start(w1rootT_s[:], w1rootT_d[:])
            b1_s = cp.tile([HID, 1], f32)
            nc.sync.dma_start(b1_s[:], b1_d[:])
            w2bothT_s = cp.tile([HID, 2 * OUT], f32)
            nc.sync.dma_start(w2bothT_s[:], w2bothT_d[:])
            b2_s = cp.tile([P, OUT], f32)
            nc.sync.dma_start(b2_s[:], b2rep_d[:])
            xT_own_s = cp.tile([IN, cfg.NPC], f32)
            nc.sync.dma_start(xT_own_s[:], xT_own_d[:])
            idxA_s = cp.tile([P, NAc * 8], i16)
            nc.sync.dma_start(idxA_s[:], idxA_d[:])
            idxB_s = cp.tile([P, NBc * 8], i16)
            nc.sync.dma_start(idxB_s[:], idxB_d[:])
            drA_s = cp.tile([P, NAc], f32)
            nc.sync.dma_start(drA_s[:], drA_d[:])
            drB_s = cp.tile([P, NBc], f32)
            nc.sync.dma_start(drB_s[:], drB_d[:])
            hroots = cp.tile([P, TPC * OUT], f32)

            hp_local = dp.tile([cfg.NPC, P], bf16)
            hp_full = dp.tile([cfg.NPAD, P], bf16)

            maxA = max(offA[t1] - offA[t0] for t0, t1 in segs)
            maxB = max(offB[t1] - offB[t0] for t0, t1 in segs)

            gq = [0]

            def layer(phase, gtabA, gtabB, consume_tile):
                """One gather+segment-sum pass over all tiles."""
                width = P if phase == 1 else 2 * OUT
                with (
                    tc.tile_pool(name=f"G{phase}", bufs=2) as gp,
                    tc.tile_pool(name=f"S{phase}", bufs=4) as sp,
                    tc.tile_pool(name=f"agg{phase}", bufs=2, space="PSUM") as ap,
                ):
                    def gather_piece(G, gtab, idx_s, base, c0, c1):
                        # round-robin the 4 SWDGE queues; each ring caps at
                        # 1024 descriptors and a lone queue stalls the ucode
                        # on ring drain.
                        nc.gpsimd.dma_gather(
                            G[:, c0:c1, :], gtab,
                            idx_s[:, (base + c0) * 8:(base + c1) * 8],
                            (c1 - c0) * P, (c1 - c0) * P, P,
                            queue_num=gq[0] % 4)
                        gq[0] += 1

                    for t0, t1 in segs:
                        a0, a1 = offA[t0], offA[t1]
                        b0, b1_ = offB[t0], offB[t1]
                        nA, nB = a1 - a0, b1_ - b0
                        # dma_gather fails above 1024 indices per call
                        # (HW ring limit) -- split into <=8-chunk pieces.
                        GMAX = 8
                        GA = gp.tile([P, maxA, P], bf16, tag="GA")
                        for c0 in range(0, nA, GMAX):
                            gather_piece(GA, gtabA, idxA_s, a0,
                                         c0, min(c0 + GMAX, nA))
                        GB = gp.tile([P, maxB, P], bf16, tag="GB")
                        for c0 in range(0, nB, GMAX):
                            gather_piece(GB, gtabB, idxB_s, b0,
                                         c0, min(c0 + GMAX, nB))
                        kmaxA = max(kA)
                        kmaxB = max(kB) if max(kB) else 1
                        for t in range(t0, t1):
                            nch = kA[t] + kB[t]
                            acc = ap.tile([P, width], f32, tag="acc")
                            ci = 0
                            for g, G, off, soff, dr, kmax in (
                                (0, GA, offA[t] - a0, offA[t], drA_s, kmaxA),
                                (1, GB, offB[t] - b0, offB[t], drB_s, kmaxB),
                            ):
                                kk = kA[t] if g == 0 else kB[t]
                                if kk == 0:
                                    continue
                                S = sp.tile([P, kmax, P], bf16, tag=f"S{g}")
                                nc.vector.tensor_tensor(
                                    out=S[:, :kk, :],
                                    in0=iota_s[:].unsqueeze(1)
                                        .to_broadcast([P, kk, P]),
                                    in1=dr[:, soff:soff + kk].unsqueeze(2)
                                        .to_broadcast([P, kk, P]),
                                    op=Alu.is_equal)
                                for j in range(kk):
                                    # acc[d, hi|lo] += sum_e S[e,d] G[e,:]
                                    # selector one-hot is exact in bf16;
                                    # hi+lo re-add recovers f32 precision.
                                    nc.tensor.matmul(
                                        acc[:], lhsT=S[:, j, :],
                                        rhs=G[:, off + j, :width],
                                        start=(ci == 0),
                                        stop=(ci == nch - 1))
                                    ci += 1
                            consume_tile(t, acc)

            # ---------------- phase 1
            with (
                tc.tile_pool(name="sb1", bufs=3) as sb1,
                tc.tile_pool(name="hps", bufs=2, space="PSUM") as hps,
            ):
                def consume1(t, acc):
                    # agg[d, f] = hi + lo halves (DVE reads at most one PSUM
                    # input: stage hi through scalar first)
                    agg_sb = sb1.tile([P, IN], f32, tag="aggds")
                    nc.scalar.activation(agg_sb[:], acc[:, :IN], Act.Copy)
                    nc.vector.tensor_tensor(
                        out=agg_sb[:], in0=agg_sb[:], in1=acc[:, IN:2 * IN],
                        op=Alu.add)
                    # transpose to [f, d] for the f32 projections
                    aggT_ps = hps.tile([IN, P], f32, tag="aggT")
                    nc.tensor.transpose(aggT_ps[:], agg_sb[:], ident_s[:])
                    aggsb = sb1.tile([IN, P], f32, tag="aggsb")
                    nc.scalar.activation(aggsb[:], aggT_ps[:], Act.Copy)
                    hT_ps = hps.tile([HID, P], f32, tag="hT")
                    nc.tensor.matmul(hT_ps[:], lhsT=w1relT_s[:], rhs=aggsb[:],
                                     start=True, stop=False)
                    nc.tensor.matmul(hT_ps[:], lhsT=w1rootT_s[:],
                                     rhs=xT_own_s[:, t * P:(t + 1) * P],
                                     start=False, stop=True)
                    hT_sb = sb1.tile([HID, P], f32, tag="hTsb")
                    nc.scalar.activation(hT_sb[:], hT_ps[:], Act.Relu,
                                         bias=b1_s[:, 0:1])
                    hh_ps = hps.tile([P, 2 * OUT], f32, tag="hh")
                    nc.tensor.matmul(hh_ps[:], lhsT=hT_sb[:],
                                     rhs=w2bothT_s[:], start=True, stop=True)
                    nc.vector.tensor_tensor(
                        out=hroots[:, t * OUT:(t + 1) * OUT],
                        in0=hh_ps[:, OUT:], in1=b2_s[:], op=Alu.add)
                    # split hp into bf16 hi/lo packed row [hi|lo|garbage]
                    hp_pack = sb1.tile([P, P], bf16, tag="hppack")
                    nc.vector.tensor_copy(out=hp_pack[:, :OUT],
                                          in_=hh_ps[:, :OUT])
                    nc.vector.tensor_tensor(
                        out=hp_pack[:, OUT:2 * OUT], in0=hh_ps[:, :OUT],
                        in1=hp_pack[:, :OUT], op=Alu.subtract)
                    nc.sync.dma_start(
                        out=hp_local[t * P:(t + 1) * P, :],
                        in_=hp_pack[:])

                # Two AllGather chunks, aligned to the A/B gather-table
                # split (a segment boundary): the A chunk (segs 0-3) fires
                # while layer 1 still computes segs 4-6, unblocking layer-2
                # A-gathers the moment layer-1 gathers drain; the B chunk
                # fires at the layer-1 tail. hp_full rows are segment-major
                # (seg_perm) so each chunk's 8-rank output is contiguous.
                segrows = cfg.SEG * P
                a_segs = cfg.SPLIT // (segrows * cfg.NCORES)
                a_local = a_segs * segrows

                def allgather_rows(r0, r1):
                    nc.gpsimd.collective_compute(
                        "AllGather", mybir.AluOpType.bypass,
                        replica_groups=[list(range(cfg.NCORES))],
                        ins=[hp_local[r0:r1, :]],
                        outs=[hp_full[r0 * cfg.NCORES:r1 * cfg.NCORES, :]],
                    )

                def consume1_and_gather(t, acc):
                    consume1(t, acc)
                    if t == segs[a_segs - 1][1] - 1:
                        allgather_rows(0, a_local)
                    elif t == TPC - 1:
                        allgather_rows(a_local, cfg.NPC)

                layer(1, x_tab[:cfg.SPLIT, :], x_tab[cfg.SPLIT:, :],
                      consume1_and_gather)

            # ---------------- phase 2
            with tc.tile_pool(name="sb2", bufs=3) as sb2:
                def consume2(t, acc):
                    # two adds, each with at most one PSUM operand
                    o1 = sb2.tile([P, OUT], f32, tag="o1")
                    nc.vector.tensor_tensor(
                        out=o1[:], in0=acc[:, :OUT],
                        in1=hroots[:, t * OUT:(t + 1) * OUT], op=Alu.add)
                    nc.vector.tensor_tensor(
                        out=o1[:], in0=o1[:], in1=acc[:, OUT:2 * OUT],
                        op=Alu.add)
                    mx = sb2.tile([P, 1], f32, tag="mx")
                    nc.vector.reduce_max(out=mx[:], in_=o1[:],
                                         axis=mybir.AxisListType.X)
                    nmx = sb2.tile([P, 1], f32, tag="nmx")
                    nc.vector.tensor_scalar(nmx[:], mx[:], -1.0, None, Alu.mult)
                    esc = sb2.tile([P, OUT], f32, tag="esc")
                    ssum = sb2.tile([P, 1], f32, tag="ssum")
                    nc.scalar.activation(esc[:], o1[:], Act.Exp,
                                         bias=nmx[:, 0:1], accum_out=ssum[:])
                    lse = sb2.tile([P, 1], f32, tag="lse")
                    nc.scalar.activation(lse[:], ssum[:], Act.Ln)
                    shift = sb2.tile([P, 1], f32, tag="shift")
                    nc.vector.tensor_tensor(out=shift[:], in0=mx[:],
                                            in1=lse[:], op=Alu.add)
                    res = sb2.tile([P, OUT], f32, tag="res")
                    nc.vector.tensor_tensor(
                        out=res[:], in0=o1[:],
                        in1=shift[:, 0:1].to_broadcast([P, OUT]),
                        op=Alu.subtract)
                    nc.sync.dma_start(out=out_d[t * P:(t + 1) * P, :],
                                      in_=res[:])

                layer(2, hp_full[:cfg.SPLIT, :], hp_full[cfg.SPLIT:, :],
                      consume2)

    nc.compile()
    return nc


# ---------------------------------------------------------------- runner

_CACHE = {}


def _get_program(cfg, kA, kB):
    key = (cfg.N, cfg.E, cfg.NCORES, cfg.TPC, tuple(kA), tuple(kB))
    if key not in _CACHE:
        _CACHE[key] = build(cfg, kA, kB)
    return _CACHE[key]


def run_gcn(inputs, cfg, trace=False):
    from concourse import bass_utils

    kA, kB, per_core = preprocess(inputs["x"], inputs["edge_index"], cfg)
    wts = make_weight_inputs(inputs["W1_rel"], inputs["b1"], inputs["W1_root"],
                             inputs["W2_rel"], inputs["b2"], inputs["W2_root"],
                             cfg)
    nc = _get_program(cfg, kA, kB)
    in_maps = []
    for c in range(cfg.NCORES):
        m = dict(per_core[c])
        m.update(wts)
        in_maps.append({k: m[k] for k in (
            "x_tab", "xT_own", "idxA", "idxB", "drA", "drB",
            "w1relT", "w1rootT", "b1", "w2bothT", "b2rep", "iota", "ident")})
    res = bass_utils.run_bass_kernel_spmd(
        nc, in_maps, core_ids=list(range(cfg.NCORES)), trace=trace)
    outs = [res.results[c]["out"] for c in range(cfg.NCORES)]
    full = np.concatenate(outs, axis=0)[:cfg.N]
    return full, res


def kernel(**inputs):
    out, _ = run_gcn(inputs, REAL_CFG)
    return out
